# revision 12
# baseline (speedup 1.0000x reference)
"""Trainium2 Bass kernel for the sparse-attention module.

Reference computation (per batch item b):
    query   = hidden @ Wh.T + bh                          [A]
    ast     = conv2d(alpha_sum, Wac, 11x11, pad 5)        [CK, H, W]
    cov     = einsum('khw,ak->hwa', ast, Waw)             [H, W, A]
    cnn_t   = einsum('chw,ac->hwa', cnn, Wec) + bec       [H, W, A]
    score   = tanh(query + cov + cnn_t)                   [H, W, A]
    energy  = score @ Wv[0] + bv                          [H, W]
    alpha   = softmax-ish(energy) * mask / (sum + 1e-10)
    ctx     = einsum('hw,chw->c', alpha, cnn)             [C]

Kernel strategy:
  * Data-parallel over batch: 32 / 8 cores = 4 batch items per core. No
    collectives: the reference's global max-subtract cancels in the softmax
    up to the +1e-10 epsilon (relative effect ~1e-12), so it is dropped.
    bv likewise shifts all energies equally and cancels.
  * Conv fusion: cov = conv(alpha_sum, Wfused) with
    Wfused[a, ij] = sum_k Waw[a, k] * Wac[k, 0, i, j]  (computed on device),
    so the CK=512-channel conv + projection collapse into a single 121-tap
    conv, evaluated as an im2col matmul (im2col built host-side - pure
    data movement, zero-padded to K=128).
  * Main matmuls in float32r (TF32-ish, 1 cycle/row on PE); all
    contractions padded to K=128 (sub-128-K matmuls measured ~1.75x
    slower on HW).
  * Score tiles live as [A-chunk=128 partitions, 512 pixels]; tanh on
    ScalarE folds the per-(batch, A-chunk) bias (query + bh + bec).
  * energy = Wv . score via PE (contraction over A on partitions).
  * Per-(batch, pixel-half) pipelined epilogue: exp on ScalarE straight
    from PSUM, masked-exp + denominator via fused DVE scalar_tensor_tensor,
    GPSIMD partition_broadcast of the unnormalized masked exp, context
    accumulated per half on DVE, normalized at the end by the scalar
    1/(den+1e-10) - so the context work overlaps the next half's matmuls.
"""

import numpy as np

import concourse.bacc as bacc
import concourse.tile as tile
from concourse import mybir
from concourse.bass_utils import run_bass_kernel_spmd

# Problem shapes (hardcoded per contract)
B, C, H, W = 32, 684, 16, 64
HID, A, CK, K = 256, 512, 512, 11
NCORES = 8
NB = B // NCORES          # batch items per core = 4
NPIX = H * W              # 1024
KK = K * K                # 121
NCC = 6                   # C chunks: 5 x 128 + 44 (padded to 128)
CREM = C - 5 * 128        # 44
NAC = A // 128            # 4 A-chunks
NHC = HID // 128          # 2

F32 = mybir.dt.float32
F32R = mybir.dt.float32r
MULT = mybir.AluOpType.mult
ADD = mybir.AluOpType.add
AF = mybir.ActivationFunctionType

LAST_RESULT = None
_PROGRAM = None


def _emit(nc, tc, d):
    """Emit the SPMD per-core program. d maps names -> DRAM APs."""
    import contextlib

    with contextlib.ExitStack() as ctx:
        const = ctx.enter_context(tc.tile_pool(name="const", bufs=1))
        apsum = ctx.enter_context(tc.tile_pool(name="apsum", bufs=1, space="PSUM"))
        mpool = ctx.enter_context(tc.tile_pool(name="mpsum", bufs=4, space="PSUM"))
        epool = ctx.enter_context(tc.tile_pool(name="epsum", bufs=2, space="PSUM"))
        spool = ctx.enter_context(tc.tile_pool(name="score", bufs=3))
        rows2 = ctx.enter_context(tc.tile_pool(name="rows2", bufs=2))
        rows1 = ctx.enter_context(tc.tile_pool(name="rows1", bufs=1))
        bpool = ctx.enter_context(tc.tile_pool(name="bcast", bufs=2))
        tpool = ctx.enter_context(tc.tile_pool(name="trash", bufs=1))
        cpool = ctx.enter_context(tc.tile_pool(name="ctxc", bufs=2))
        maskp = ctx.enter_context(tc.tile_pool(name="maskp", bufs=2))
        asump = ctx.enter_context(tc.tile_pool(name="asump", bufs=2))

        # ---- weights needed by the prep matmuls go first (sync queue) ----
        wawt = const.tile([128, 4, A], F32R)       # Waw.T, k-chunked
        nc.sync.dma_start(
            out=wawt,
            in_=d["wawt"][:, :].rearrange("(kc k) a -> k kc a", k=128).bitcast(F32R),
        )
        wacf = const.tile([128, 4, 128], F32R)     # Wac flat (K-padded), k-chunked
        nc.sync.dma_start(
            out=wacf,
            in_=d["wacf"][:, :].rearrange("(kc k) t -> k kc t", k=128).bitcast(F32R),
        )

        # ---- small weights on the scalar queue (parallel with bulk) ----
        wht = const.tile([128, NHC, A], F32)       # Wh.T, hid-chunked
        nc.scalar.dma_start(
            out=wht, in_=d["wht"][:, :].rearrange("(hc h) a -> h hc a", h=128)
        )
        hid = const.tile([128, NHC, NB], F32)      # hidden.T, hid-chunked
        nc.scalar.dma_start(
            out=hid, in_=d["hiddent"][:, :].rearrange("(hc h) b -> h hc b", h=128)
        )
        wv = const.tile([128, NAC], F32R)          # Wv as [p, ac]
        nc.scalar.dma_start(
            out=wv, in_=d["wv"][:].rearrange("(ac p) -> p ac", p=128).bitcast(F32R)
        )
        bhv = const.tile([128, NAC], F32)
        nc.scalar.dma_start(out=bhv, in_=d["bh"][:].rearrange("(ac p) -> p ac", p=128))
        becv = const.tile([128, NAC], F32)
        nc.scalar.dma_start(out=becv, in_=d["bec"][:].rearrange("(ac p) -> p ac", p=128))

        # ---- Wec.T (host-padded to 768 rows): 6 uniform K=128 chunks ----
        wect = const.tile([128, NCC, A], F32R)
        nc.sync.dma_start(
            out=wect,
            in_=d["wect"][:, :].rearrange("(cc c) a -> c cc a", c=128).bitcast(F32R),
        )

        # ---- feature tiles: one merged DMA per batch item (DMA issue ops
        #      cost ~0.7us each, so fewer+bigger wins); batch 0 is split by
        #      pixel half so the first matmul group's data lands early ----
        cnn_b0h = [const.tile([128, NCC, 512], F32R, tag=f"cnn0h{h}",
                              name=f"cnn0h{h}") for h in range(2)]
        im2_b0h = [const.tile([128, 512], F32R, tag=f"im20h{h}",
                              name=f"im20h{h}") for h in range(2)]
        cnn_bt = [None] + [const.tile([128, NCC, NPIX], F32R, tag=f"cnn{b}",
                                      name=f"cnn{b}") for b in range(1, NB)]
        im2_bt = [None] + [const.tile([128, NPIX], F32R, tag=f"im2_{b}",
                                      name=f"im2_{b}") for b in range(1, NB)]

        def cnn_ap(b, cc, h):
            """rhs AP for (batch, C-chunk, pixel-half)."""
            if b == 0:
                return cnn_b0h[h][:, cc, :]
            return cnn_bt[b][:, cc, h * 512:(h + 1) * 512]

        def im2_ap(b, h):
            if b == 0:
                return im2_b0h[h]
            return im2_bt[b][:, h * 512:(h + 1) * 512]

        for h in range(2):
            nc.sync.dma_start(
                out=im2_b0h[h],
                in_=d["im2col"][0, :, h * 512:(h + 1) * 512].bitcast(F32R),
            )
            nc.sync.dma_start(
                out=cnn_b0h[h],
                in_=d["cnn"][0, :, h * 512:(h + 1) * 512]
                .rearrange("(cc c) p -> c cc p", c=128).bitcast(F32R),
            )

        # ---- prep compute: fused conv weight + per-batch bias vectors ----
        # WfusedT[t, a] = sum_k Wac_flat[k, t] * WawT[k, a]; rows 121..127
        # are zero because the padded wacf columns are zero.
        wf_ps = apsum.tile([128, A], F32)
        for kc in range(4):
            nc.tensor.matmul(
                wf_ps, lhsT=wacf[:, kc, :], rhs=wawt[:, kc, :],
                start=(kc == 0), stop=(kc == 3),
            )
        wft = const.tile([128, A], F32R)
        nc.vector.tensor_copy(wft, wf_ps)

        # bias bb[p, ac] = bh + bec ; qb[p, ac, b] = (hidden @ Wh.T)[b, a] + bb
        bb = const.tile([128, NAC], F32)
        nc.vector.tensor_add(bb, bhv, becv)
        qb = const.tile([128, NAC, NB], F32)
        for ac in range(NAC):
            q_ps = apsum.tile([128, NB], F32, tag="qps")
            for hc in range(NHC):
                nc.tensor.matmul(
                    q_ps, lhsT=wht[:, hc, ac * 128:(ac + 1) * 128],
                    rhs=hid[:, hc, :], start=(hc == 0), stop=(hc == NHC - 1),
                )
            nc.vector.tensor_scalar_add(qb[:, ac, :], q_ps, bb[:, ac:ac + 1])

        # ---- bulk loads for remaining batch items (sync queue) ----
        for b in range(1, NB):
            nc.sync.dma_start(out=im2_bt[b], in_=d["im2col"][b, :, :].bitcast(F32R))
            nc.sync.dma_start(
                out=cnn_bt[b],
                in_=d["cnn"][b, :, :].rearrange("(cc c) p -> c cc p", c=128)
                .bitcast(F32R),
            )

        # ---- main loop ----
        for b in range(NB):
            mask_row = maskp.tile([1, NPIX], F32, tag="mask")
            nc.sync.dma_start(out=mask_row, in_=d["mask"][b, :])
            asum_row = asump.tile([1, NPIX], F32, tag="asum_in")
            nc.sync.dma_start(out=asum_row, in_=d["asum"][b, :])
            em_row = rows2.tile([1, NPIX], F32, tag="em")
            dens = rows1.tile([1, 2], F32, tag="dens")
            ctxc = cpool.tile([128, NCC, 2], F32, tag="ctxc")
            for h in range(2):
                hsl = slice(h * 512, (h + 1) * 512)
                e_ps = epool.tile([1, 512], F32, tag="eps")
                for ac in range(NAC):
                    asl = slice(ac * 128, (ac + 1) * 128)
                    ps = mpool.tile([128, 512], F32, tag="mps")
                    for cc in range(NCC):
                        nc.tensor.matmul(
                            ps, lhsT=wect[:, cc, asl],
                            rhs=cnn_ap(b, cc, h),
                            start=(cc == 0), stop=False,
                        )
                    nc.tensor.matmul(
                        ps, lhsT=wft[:, asl], rhs=im2_ap(b, h),
                        start=False, stop=True,
                    )
                    sc = spool.tile([128, 512], F32R, tag="sc")
                    nc.scalar.activation(
                        sc, ps, AF.Tanh, bias=qb[:, ac, b:b + 1], scale=1.0
                    )
                    nc.tensor.matmul(
                        e_ps, lhsT=wv[:, ac:ac + 1], rhs=sc,
                        start=(ac == 0), stop=(ac == NAC - 1),
                    )
                # energy -> exp (no max-subtract needed; |energy| is O(1))
                exp_h = rows2.tile([1, 512], F32, tag="exph")
                nc.scalar.activation(exp_h, e_ps, AF.Exp)
                # masked exp + this half's denominator (fused)
                nc.vector.scalar_tensor_tensor(
                    out=em_row[:, hsl], in0=exp_h, scalar=1.0,
                    in1=mask_row[:, hsl], op0=MULT, op1=MULT,
                    accum_out=dens[:, h:h + 1],
                )
                # unnormalized context for this half (overlaps next half's PE)
                ab = bpool.tile([128, 512], F32, tag="ab")
                nc.gpsimd.partition_broadcast(ab, em_row[:, hsl])
                trash = tpool.tile([128, 512], F32, tag="trash")
                for cc in range(NCC):
                    nc.vector.scalar_tensor_tensor(
                        out=trash, in0=cnn_ap(b, cc, h).bitcast(F32),
                        scalar=1.0, in1=ab, op0=MULT, op1=MULT,
                        accum_out=ctxc[:, cc, h:h + 1],
                    )

            # denominator and 1/(den + 1e-10)
            den = rows1.tile([1, 1], F32, tag="den")
            nc.vector.tensor_scalar_add(den, dens[:, 0:1], dens[:, 1:2])
            nc.vector.tensor_scalar_add(den, den, 1e-10)
            rcp = rows1.tile([1, 1], F32, tag="rcp")
            nc.vector.reciprocal(rcp, den)

            # alpha = em * rcp ; alpha_sum_new = alpha + alpha_sum (in place)
            alpha_row = rows2.tile([1, NPIX], F32, tag="alpha")
            nc.vector.tensor_scalar_mul(alpha_row, em_row, rcp)
            nc.sync.dma_start(out=d["alpha"][b, :], in_=alpha_row)
            nc.vector.tensor_add(asum_row, alpha_row, asum_row)
            nc.sync.dma_start(out=d["asum_new"][b, :], in_=asum_row)

            # context: combine halves, scale by rcp (broadcast to partitions)
            rcp128 = rows1.tile([128, 1], F32, tag="rcp128")
            nc.gpsimd.partition_broadcast(rcp128, rcp)
            ctx_fin = cpool.tile([128, NCC], F32, tag="ctxfin")
            nc.vector.tensor_add(ctx_fin, ctxc[:, :, 0], ctxc[:, :, 1])
            nc.vector.tensor_scalar_mul(ctx_fin, ctx_fin, rcp128)
            nc.sync.dma_start(
                out=d["ctx"][b, 0:640].rearrange("(cc p) -> p cc", p=128),
                in_=ctx_fin[:, 0:5],
            )
            nc.sync.dma_start(out=d["ctx"][b, 640:C], in_=ctx_fin[0:CREM, 5:6])


def _build():
    nc = bacc.Bacc(
        "TRN2", target_bir_lowering=False, debug=False, enable_asserts=False
    )
    d = {}
    def inp(name, shape):
        d[name] = nc.dram_tensor(name, list(shape), F32, kind="ExternalInput")[:]
    def outp(name, shape):
        d[name] = nc.dram_tensor(name, list(shape), F32, kind="ExternalOutput")[:]

    inp("cnn", (NB, 768, NPIX))
    inp("im2col", (NB, 128, NPIX))
    inp("hiddent", (HID, NB))
    inp("mask", (NB, NPIX))
    inp("asum", (NB, NPIX))
    inp("wect", (768, A))
    inp("wht", (HID, A))
    inp("wawt", (CK, A))
    inp("wacf", (CK, 128))
    inp("wv", (A,))
    inp("bh", (A,))
    inp("bec", (A,))
    outp("ctx", (NB, C))
    outp("alpha", (NB, NPIX))
    outp("asum_new", (NB, NPIX))

    with tile.TileContext(nc) as tc:
        _emit(nc, tc, d)
    nc.compile()
    return nc


def get_program():
    global _PROGRAM
    if _PROGRAM is None:
        _PROGRAM = _build()
    return _PROGRAM


def _prep_in_maps(cnn_features, hidden, alpha_sum, image_mask,
                  Wh, bh, Wec, bec, Wac, Waw, Wv, bv):
    f = np.float32
    cnn = np.zeros((B, 768, NPIX), dtype=f)
    cnn[:, :C, :] = np.asarray(cnn_features, dtype=f).reshape(B, C, NPIX)
    asum = np.ascontiguousarray(alpha_sum, dtype=f).reshape(B, NPIX)
    mask = np.ascontiguousarray(image_mask, dtype=f).reshape(B, NPIX)
    # host im2col: pure data rearrangement of alpha_sum (zero-padded windows),
    # K-padded from 121 to 128 rows with zeros
    pad = np.zeros((B, H + K - 1, W + K - 1), dtype=f)
    pad[:, K // 2:K // 2 + H, K // 2:K // 2 + W] = asum.reshape(B, H, W)
    win = np.lib.stride_tricks.sliding_window_view(pad, (H, W), axis=(1, 2))
    im2 = np.zeros((B, 128, NPIX), dtype=f)
    im2[:, :KK, :] = win.reshape(B, KK, NPIX)
    wect_pad = np.zeros((768, A), dtype=f)
    wect_pad[:C, :] = np.asarray(Wec, dtype=f).T
    # Wac flat, K-padded 121 -> 128 with zero columns
    wacf = np.zeros((CK, 128), dtype=f)
    wacf[:, :KK] = np.asarray(Wac, dtype=f).reshape(CK, KK)

    shared = {
        "wect": wect_pad,
        "wht": np.ascontiguousarray(np.asarray(Wh, dtype=f).T),
        "wawt": np.ascontiguousarray(np.asarray(Waw, dtype=f).T),
        "wacf": wacf,
        "wv": np.ascontiguousarray(np.asarray(Wv, dtype=f).reshape(A)),
        "bh": np.ascontiguousarray(np.asarray(bh, dtype=f)),
        "bec": np.ascontiguousarray(np.asarray(bec, dtype=f)),
    }
    hiddenT = np.ascontiguousarray(np.asarray(hidden, dtype=f).T)  # [HID, B]
    in_maps = []
    for m in range(NCORES):
        sl = slice(m * NB, (m + 1) * NB)
        in_maps.append({
            "cnn": np.ascontiguousarray(cnn[sl]),
            "im2col": np.ascontiguousarray(im2[sl]),
            "hiddent": np.ascontiguousarray(hiddenT[:, sl]),
            "mask": np.ascontiguousarray(mask[sl]),
            "asum": np.ascontiguousarray(asum[sl]),
            **shared,
        })
    return in_maps


def kernel(cnn_features, hidden, alpha_sum, image_mask,
           Wh, bh, Wec, bec, Wac, Waw, Wv, bv):
    global LAST_RESULT
    nc = get_program()
    in_maps = _prep_in_maps(cnn_features, hidden, alpha_sum, image_mask,
                            Wh, bh, Wec, bec, Wac, Waw, Wv, bv)
    res = run_bass_kernel_spmd(nc, in_maps, list(range(NCORES)))
    LAST_RESULT = res
    ctx = np.concatenate([res.results[m]["ctx"] for m in range(NCORES)], axis=0)
    alpha = np.concatenate(
        [res.results[m]["alpha"] for m in range(NCORES)], axis=0
    ).reshape(B, H, W)
    asum_new = np.concatenate(
        [res.results[m]["asum_new"] for m in range(NCORES)], axis=0
    ).reshape(B, 1, H, W)
    return (ctx.astype(np.float32), alpha.astype(np.float32),
            asum_new.astype(np.float32))


# revision 13
# speedup vs baseline: 1.1033x; 1.1033x over previous
"""Trainium2 Bass kernel for the sparse-attention module.

Reference computation (per batch item b):
    query   = hidden @ Wh.T + bh                          [A]
    ast     = conv2d(alpha_sum, Wac, 11x11, pad 5)        [CK, H, W]
    cov     = einsum('khw,ak->hwa', ast, Waw)             [H, W, A]
    cnn_t   = einsum('chw,ac->hwa', cnn, Wec) + bec       [H, W, A]
    score   = tanh(query + cov + cnn_t)                   [H, W, A]
    energy  = score @ Wv[0] + bv                          [H, W]
    alpha   = softmax-ish(energy) * mask / (sum + 1e-10)
    ctx     = einsum('hw,chw->c', alpha, cnn)             [C]

Kernel strategy:
  * Data-parallel over batch: 32 / 8 cores = 4 batch items per core. No
    collectives: the reference's global max-subtract cancels in the softmax
    up to the +1e-10 epsilon (relative effect ~1e-12), so it is dropped.
    bv likewise shifts all energies equally and cancels.
  * Conv fusion: cov = conv(alpha_sum, Wfused) with
    Wfused[a, ij] = sum_k Waw[a, k] * Wac[k, 0, i, j]  (computed on device),
    so the CK=512-channel conv + projection collapse into a single 121-tap
    conv, evaluated as an im2col matmul (im2col built host-side - pure
    data movement, zero-padded to K=128).
  * Main matmuls in float32r (TF32-ish, 1 cycle/row on PE); all
    contractions padded to K=128 (sub-128-K matmuls measured ~1.75x
    slower on HW).
  * Score tiles live as [A-chunk=128 partitions, 512 pixels]; tanh on
    ScalarE folds the per-(batch, A-chunk) bias (query + bh + bec).
  * energy = Wv . score via PE (contraction over A on partitions).
  * Per-(batch, pixel-half) pipelined epilogue: exp on ScalarE straight
    from PSUM, masked-exp + denominator via fused DVE scalar_tensor_tensor,
    GPSIMD partition_broadcast of the unnormalized masked exp, context
    accumulated per half on DVE, normalized at the end by the scalar
    1/(den+1e-10) - so the context work overlaps the next half's matmuls.
"""

import numpy as np

import concourse.bacc as bacc
import concourse.tile as tile
from concourse import mybir
from concourse.bass_utils import run_bass_kernel_spmd

# Problem shapes (hardcoded per contract)
B, C, H, W = 32, 684, 16, 64
HID, A, CK, K = 256, 512, 512, 11
NCORES = 8
NB = B // NCORES          # batch items per core = 4
NPIX = H * W              # 1024
KK = K * K                # 121
NCC = 6                   # C chunks: 5 x 128 + 44 (padded to 128)
CREM = C - 5 * 128        # 44
NAC = A // 128            # 4 A-chunks
NHC = HID // 128          # 2

F32 = mybir.dt.float32
F32R = mybir.dt.float32r
MULT = mybir.AluOpType.mult
ADD = mybir.AluOpType.add
AF = mybir.ActivationFunctionType

LAST_RESULT = None
_PROGRAM = None


def _emit(nc, tc, d):
    """Emit the SPMD per-core program. d maps names -> DRAM APs."""
    import contextlib

    with contextlib.ExitStack() as ctx:
        const = ctx.enter_context(tc.tile_pool(name="const", bufs=1))
        apsum = ctx.enter_context(tc.tile_pool(name="apsum", bufs=1, space="PSUM"))
        mpool = ctx.enter_context(tc.tile_pool(name="mpsum", bufs=4, space="PSUM"))
        epool = ctx.enter_context(tc.tile_pool(name="epsum", bufs=2, space="PSUM"))
        spool = ctx.enter_context(tc.tile_pool(name="score", bufs=3))
        rows2 = ctx.enter_context(tc.tile_pool(name="rows2", bufs=2))
        rows1 = ctx.enter_context(tc.tile_pool(name="rows1", bufs=1))
        bpool = ctx.enter_context(tc.tile_pool(name="bcast", bufs=2))
        tpool = ctx.enter_context(tc.tile_pool(name="trash", bufs=1))
        cpool = ctx.enter_context(tc.tile_pool(name="ctxc", bufs=2))
        maskp = ctx.enter_context(tc.tile_pool(name="maskp", bufs=2))
        asump = ctx.enter_context(tc.tile_pool(name="asump", bufs=2))

        # ---- weights needed by the prep matmuls go first (sync queue) ----
        wawt = const.tile([128, 4, A], F32R)       # Waw.T, k-chunked
        nc.sync.dma_start(
            out=wawt,
            in_=d["wawt"][:, :].rearrange("(kc k) a -> k kc a", k=128).bitcast(F32R),
        )
        wacf = const.tile([128, 4, 128], F32R)     # Wac flat (K-padded), k-chunked
        nc.sync.dma_start(
            out=wacf,
            in_=d["wacf"][:, :].rearrange("(kc k) t -> k kc t", k=128).bitcast(F32R),
        )

        # ---- small weights on the scalar queue (parallel with bulk) ----
        wht = const.tile([128, NHC, A], F32)       # Wh.T, hid-chunked
        nc.scalar.dma_start(
            out=wht, in_=d["wht"][:, :].rearrange("(hc h) a -> h hc a", h=128)
        )
        hid = const.tile([128, NHC, NB], F32)      # hidden.T, hid-chunked
        nc.scalar.dma_start(
            out=hid, in_=d["hiddent"][:, :].rearrange("(hc h) b -> h hc b", h=128)
        )
        wv = const.tile([128, NAC], F32R)          # Wv as [p, ac]
        nc.scalar.dma_start(
            out=wv, in_=d["wv"][:].rearrange("(ac p) -> p ac", p=128).bitcast(F32R)
        )
        bhv = const.tile([128, NAC], F32)
        nc.scalar.dma_start(out=bhv, in_=d["bh"][:].rearrange("(ac p) -> p ac", p=128))
        becv = const.tile([128, NAC], F32)
        nc.scalar.dma_start(out=becv, in_=d["bec"][:].rearrange("(ac p) -> p ac", p=128))

        # ---- Wec.T (host-padded to 768 rows): 6 uniform K=128 chunks ----
        wect = const.tile([128, NCC, A], F32R)
        nc.sync.dma_start(
            out=wect,
            in_=d["wect"][:, :].rearrange("(cc c) a -> c cc a", c=128).bitcast(F32R),
        )

        # ---- per-(b, cc) single-writer feature tiles; b0 split across both
        #      DMA queues so the first matmul group's data lands early ----
        cnn_t = [[const.tile([128, NPIX], F32R, tag=f"cnn{b}_{cc}",
                             name=f"cnn{b}_{cc}") for cc in range(NCC)]
                 for b in range(NB)]
        im2_t = [const.tile([128, NPIX], F32R, tag=f"im2_{b}", name=f"im2_{b}")
                 for b in range(NB)]

        def load_batch_data(b):
            nc.sync.dma_start(out=im2_t[b], in_=d["im2col"][b, :, :].bitcast(F32R))
            for cc in range(NCC):
                nc.sync.dma_start(
                    out=cnn_t[b][cc],
                    in_=d["cnn"][b, cc * 128:(cc + 1) * 128, :].bitcast(F32R),
                )

        load_batch_data(0)

        # ---- prep compute: fused conv weight + per-batch bias vectors ----
        # WfusedT[t, a] = sum_k Wac_flat[k, t] * WawT[k, a]; rows 121..127
        # are zero because the padded wacf columns are zero.
        wf_ps = apsum.tile([128, A], F32)
        for kc in range(4):
            nc.tensor.matmul(
                wf_ps, lhsT=wacf[:, kc, :], rhs=wawt[:, kc, :],
                start=(kc == 0), stop=(kc == 3),
            )
        wft = const.tile([128, A], F32R)
        nc.vector.tensor_copy(wft, wf_ps)

        # bias bb[p, ac] = bh + bec ; qb[p, ac, b] = (hidden @ Wh.T)[b, a] + bb
        bb = const.tile([128, NAC], F32)
        nc.vector.tensor_add(bb, bhv, becv)
        qb = const.tile([128, NAC, NB], F32)
        for ac in range(NAC):
            q_ps = apsum.tile([128, NB], F32, tag="qps")
            for hc in range(NHC):
                nc.tensor.matmul(
                    q_ps, lhsT=wht[:, hc, ac * 128:(ac + 1) * 128],
                    rhs=hid[:, hc, :], start=(hc == 0), stop=(hc == NHC - 1),
                )
            nc.vector.tensor_scalar_add(qb[:, ac, :], q_ps, bb[:, ac:ac + 1])

        # ---- bulk loads for remaining batch items (sync queue) ----
        for b in range(1, NB):
            load_batch_data(b)

        # ---- main loop ----
        for b in range(NB):
            mask_row = maskp.tile([1, NPIX], F32, tag="mask")
            nc.gpsimd.dma_start(out=mask_row, in_=d["mask"][b, :])
            asum_row = asump.tile([1, NPIX], F32, tag="asum_in")
            nc.gpsimd.dma_start(out=asum_row, in_=d["asum"][b, :])
            em_row = rows2.tile([1, NPIX], F32, tag="em")
            dens = rows1.tile([1, 2], F32, tag="dens")
            ctxc = cpool.tile([128, NCC, 2], F32, tag="ctxc")
            for h in range(2):
                hsl = slice(h * 512, (h + 1) * 512)
                e_ps = epool.tile([1, 512], F32, tag="eps")
                for ac in range(NAC):
                    asl = slice(ac * 128, (ac + 1) * 128)
                    ps = mpool.tile([128, 512], F32, tag="mps")
                    for cc in range(NCC):
                        nc.tensor.matmul(
                            ps, lhsT=wect[:, cc, asl],
                            rhs=cnn_t[b][cc][:, hsl],
                            start=(cc == 0), stop=False,
                        )
                    nc.tensor.matmul(
                        ps, lhsT=wft[:, asl], rhs=im2_t[b][:, hsl],
                        start=False, stop=True,
                    )
                    sc = spool.tile([128, 512], F32R, tag="sc")
                    nc.scalar.activation(
                        sc, ps, AF.Tanh, bias=qb[:, ac, b:b + 1], scale=1.0
                    )
                    nc.tensor.matmul(
                        e_ps, lhsT=wv[:, ac:ac + 1], rhs=sc,
                        start=(ac == 0), stop=(ac == NAC - 1),
                    )
                # energy -> exp (no max-subtract needed; |energy| is O(1))
                exp_h = rows2.tile([1, 512], F32, tag="exph")
                nc.scalar.activation(exp_h, e_ps, AF.Exp)
                # masked exp + this half's denominator (fused)
                nc.vector.scalar_tensor_tensor(
                    out=em_row[:, hsl], in0=exp_h, scalar=1.0,
                    in1=mask_row[:, hsl], op0=MULT, op1=MULT,
                    accum_out=dens[:, h:h + 1],
                )
                # unnormalized context for this half (overlaps next half's PE)
                ab = bpool.tile([128, 512], F32, tag="ab")
                nc.gpsimd.partition_broadcast(ab, em_row[:, hsl])
                trash = tpool.tile([128, 512], F32, tag="trash")
                for cc in range(NCC):
                    nc.vector.scalar_tensor_tensor(
                        out=trash, in0=cnn_t[b][cc][:, hsl].bitcast(F32),
                        scalar=1.0, in1=ab, op0=MULT, op1=MULT,
                        accum_out=ctxc[:, cc, h:h + 1],
                    )

            # denominator and 1/(den + 1e-10)
            den = rows1.tile([1, 1], F32, tag="den")
            nc.vector.tensor_scalar_add(den, dens[:, 0:1], dens[:, 1:2])
            nc.vector.tensor_scalar_add(den, den, 1e-10)
            rcp = rows1.tile([1, 1], F32, tag="rcp")
            nc.vector.reciprocal(rcp, den)

            # alpha = em * rcp ; alpha_sum_new = alpha + alpha_sum (in place)
            alpha_row = rows2.tile([1, NPIX], F32, tag="alpha")
            nc.vector.tensor_scalar_mul(alpha_row, em_row, rcp)
            nc.scalar.dma_start(out=d["alpha"][b, :], in_=alpha_row)
            nc.vector.tensor_add(asum_row, alpha_row, asum_row)
            nc.scalar.dma_start(out=d["asum_new"][b, :], in_=asum_row)

            # context: combine halves, scale by rcp (broadcast to partitions)
            rcp128 = rows1.tile([128, 1], F32, tag="rcp128")
            nc.gpsimd.partition_broadcast(rcp128, rcp)
            ctx_fin = cpool.tile([128, NCC], F32, tag="ctxfin")
            nc.vector.tensor_add(ctx_fin, ctxc[:, :, 0], ctxc[:, :, 1])
            nc.vector.tensor_scalar_mul(ctx_fin, ctx_fin, rcp128)
            nc.scalar.dma_start(
                out=d["ctx"][b, 0:640].rearrange("(cc p) -> p cc", p=128),
                in_=ctx_fin[:, 0:5],
            )
            nc.scalar.dma_start(out=d["ctx"][b, 640:C], in_=ctx_fin[0:CREM, 5:6])


def _build():
    nc = bacc.Bacc(
        "TRN2", target_bir_lowering=False, debug=False, enable_asserts=False
    )
    d = {}
    def inp(name, shape):
        d[name] = nc.dram_tensor(name, list(shape), F32, kind="ExternalInput")[:]
    def outp(name, shape):
        d[name] = nc.dram_tensor(name, list(shape), F32, kind="ExternalOutput")[:]

    inp("cnn", (NB, 768, NPIX))
    inp("im2col", (NB, 128, NPIX))
    inp("hiddent", (HID, NB))
    inp("mask", (NB, NPIX))
    inp("asum", (NB, NPIX))
    inp("wect", (768, A))
    inp("wht", (HID, A))
    inp("wawt", (CK, A))
    inp("wacf", (CK, 128))
    inp("wv", (A,))
    inp("bh", (A,))
    inp("bec", (A,))
    outp("ctx", (NB, C))
    outp("alpha", (NB, NPIX))
    outp("asum_new", (NB, NPIX))

    with tile.TileContext(nc) as tc:
        _emit(nc, tc, d)
    nc.compile()
    return nc


def get_program():
    global _PROGRAM
    if _PROGRAM is None:
        _PROGRAM = _build()
    return _PROGRAM


def _prep_in_maps(cnn_features, hidden, alpha_sum, image_mask,
                  Wh, bh, Wec, bec, Wac, Waw, Wv, bv):
    f = np.float32
    cnn = np.zeros((B, 768, NPIX), dtype=f)
    cnn[:, :C, :] = np.asarray(cnn_features, dtype=f).reshape(B, C, NPIX)
    asum = np.ascontiguousarray(alpha_sum, dtype=f).reshape(B, NPIX)
    mask = np.ascontiguousarray(image_mask, dtype=f).reshape(B, NPIX)
    # host im2col: pure data rearrangement of alpha_sum (zero-padded windows),
    # K-padded from 121 to 128 rows with zeros
    pad = np.zeros((B, H + K - 1, W + K - 1), dtype=f)
    pad[:, K // 2:K // 2 + H, K // 2:K // 2 + W] = asum.reshape(B, H, W)
    win = np.lib.stride_tricks.sliding_window_view(pad, (H, W), axis=(1, 2))
    im2 = np.zeros((B, 128, NPIX), dtype=f)
    im2[:, :KK, :] = win.reshape(B, KK, NPIX)
    wect_pad = np.zeros((768, A), dtype=f)
    wect_pad[:C, :] = np.asarray(Wec, dtype=f).T
    # Wac flat, K-padded 121 -> 128 with zero columns
    wacf = np.zeros((CK, 128), dtype=f)
    wacf[:, :KK] = np.asarray(Wac, dtype=f).reshape(CK, KK)

    shared = {
        "wect": wect_pad,
        "wht": np.ascontiguousarray(np.asarray(Wh, dtype=f).T),
        "wawt": np.ascontiguousarray(np.asarray(Waw, dtype=f).T),
        "wacf": wacf,
        "wv": np.ascontiguousarray(np.asarray(Wv, dtype=f).reshape(A)),
        "bh": np.ascontiguousarray(np.asarray(bh, dtype=f)),
        "bec": np.ascontiguousarray(np.asarray(bec, dtype=f)),
    }
    hiddenT = np.ascontiguousarray(np.asarray(hidden, dtype=f).T)  # [HID, B]
    in_maps = []
    for m in range(NCORES):
        sl = slice(m * NB, (m + 1) * NB)
        in_maps.append({
            "cnn": np.ascontiguousarray(cnn[sl]),
            "im2col": np.ascontiguousarray(im2[sl]),
            "hiddent": np.ascontiguousarray(hiddenT[:, sl]),
            "mask": np.ascontiguousarray(mask[sl]),
            "asum": np.ascontiguousarray(asum[sl]),
            **shared,
        })
    return in_maps


def kernel(cnn_features, hidden, alpha_sum, image_mask,
           Wh, bh, Wec, bec, Wac, Waw, Wv, bv):
    global LAST_RESULT
    nc = get_program()
    in_maps = _prep_in_maps(cnn_features, hidden, alpha_sum, image_mask,
                            Wh, bh, Wec, bec, Wac, Waw, Wv, bv)
    res = run_bass_kernel_spmd(nc, in_maps, list(range(NCORES)))
    LAST_RESULT = res
    ctx = np.concatenate([res.results[m]["ctx"] for m in range(NCORES)], axis=0)
    alpha = np.concatenate(
        [res.results[m]["alpha"] for m in range(NCORES)], axis=0
    ).reshape(B, H, W)
    asum_new = np.concatenate(
        [res.results[m]["asum_new"] for m in range(NCORES)], axis=0
    ).reshape(B, 1, H, W)
    return (ctx.astype(np.float32), alpha.astype(np.float32),
            asum_new.astype(np.float32))


# revision 14
# speedup vs baseline: 1.1169x; 1.0123x over previous
"""Trainium2 Bass kernel for the sparse-attention module.

Reference computation (per batch item b):
    query   = hidden @ Wh.T + bh                          [A]
    ast     = conv2d(alpha_sum, Wac, 11x11, pad 5)        [CK, H, W]
    cov     = einsum('khw,ak->hwa', ast, Waw)             [H, W, A]
    cnn_t   = einsum('chw,ac->hwa', cnn, Wec) + bec       [H, W, A]
    score   = tanh(query + cov + cnn_t)                   [H, W, A]
    energy  = score @ Wv[0] + bv                          [H, W]
    alpha   = softmax-ish(energy) * mask / (sum + 1e-10)
    ctx     = einsum('hw,chw->c', alpha, cnn)             [C]

Kernel strategy:
  * Data-parallel over batch: 32 / 8 cores = 4 batch items per core. No
    collectives: the reference's global max-subtract cancels in the softmax
    up to the +1e-10 epsilon (relative effect ~1e-12), so it is dropped.
    bv likewise shifts all energies equally and cancels.
  * Conv fusion: cov = conv(alpha_sum, Wfused) with
    Wfused[a, ij] = sum_k Waw[a, k] * Wac[k, 0, i, j]  (computed on device),
    so the CK=512-channel conv + projection collapse into a single 121-tap
    conv, evaluated as an im2col matmul (im2col built host-side - pure
    data movement, zero-padded to K=128).
  * Main matmuls in float32r (TF32-ish, 1 cycle/row on PE); all
    contractions padded to K=128 (sub-128-K matmuls measured ~1.75x
    slower on HW).
  * Score tiles live as [A-chunk=128 partitions, 512 pixels]; tanh on
    ScalarE folds the per-(batch, A-chunk) bias (query + bh + bec).
  * energy = Wv . score via PE (contraction over A on partitions).
  * Per-(batch, pixel-half) pipelined epilogue: exp on ScalarE straight
    from PSUM, masked-exp + denominator via fused DVE scalar_tensor_tensor,
    GPSIMD partition_broadcast of the unnormalized masked exp, context
    accumulated per half on DVE, normalized at the end by the scalar
    1/(den+1e-10) - so the context work overlaps the next half's matmuls.
"""

import numpy as np

import concourse.bacc as bacc
import concourse.tile as tile
from concourse import mybir
from concourse.bass_utils import run_bass_kernel_spmd

# Problem shapes (hardcoded per contract)
B, C, H, W = 32, 684, 16, 64
HID, A, CK, K = 256, 512, 512, 11
NCORES = 8
NB = B // NCORES          # batch items per core = 4
NPIX = H * W              # 1024
KK = K * K                # 121
NCC = 6                   # C chunks: 5 x 128 + 44 (padded to 128)
CREM = C - 5 * 128        # 44
NAC = A // 128            # 4 A-chunks
NHC = HID // 128          # 2

F32 = mybir.dt.float32
F32R = mybir.dt.float32r
MULT = mybir.AluOpType.mult
ADD = mybir.AluOpType.add
AF = mybir.ActivationFunctionType

LAST_RESULT = None
_PROGRAM = None


def _emit(nc, tc, d):
    """Emit the SPMD per-core program. d maps names -> DRAM APs."""
    import contextlib

    with contextlib.ExitStack() as ctx:
        const = ctx.enter_context(tc.tile_pool(name="const", bufs=1))
        apsum = ctx.enter_context(tc.tile_pool(name="apsum", bufs=1, space="PSUM"))
        mpool = ctx.enter_context(tc.tile_pool(name="mpsum", bufs=5, space="PSUM"))
        epool = ctx.enter_context(tc.tile_pool(name="epsum", bufs=1, space="PSUM"))
        spool = ctx.enter_context(tc.tile_pool(name="score", bufs=3))
        rows2 = ctx.enter_context(tc.tile_pool(name="rows2", bufs=2))
        rows1 = ctx.enter_context(tc.tile_pool(name="rows1", bufs=1))
        bpool = ctx.enter_context(tc.tile_pool(name="bcast", bufs=2))
        tpool = ctx.enter_context(tc.tile_pool(name="trash", bufs=1))
        cpool = ctx.enter_context(tc.tile_pool(name="ctxc", bufs=2))
        maskp = ctx.enter_context(tc.tile_pool(name="maskp", bufs=2))
        asump = ctx.enter_context(tc.tile_pool(name="asump", bufs=2))

        # ---- weights needed by the prep matmuls go first (sync queue) ----
        wawt = const.tile([128, 4, A], F32R)       # Waw.T, k-chunked
        nc.sync.dma_start(
            out=wawt,
            in_=d["wawt"][:, :].rearrange("(kc k) a -> k kc a", k=128).bitcast(F32R),
        )
        wacf = const.tile([128, 4, 128], F32R)     # Wac flat (K-padded), k-chunked
        nc.sync.dma_start(
            out=wacf,
            in_=d["wacf"][:, :].rearrange("(kc k) t -> k kc t", k=128).bitcast(F32R),
        )

        # ---- small weights on the scalar queue (parallel with bulk) ----
        wht = const.tile([128, NHC, A], F32)       # Wh.T, hid-chunked
        nc.scalar.dma_start(
            out=wht, in_=d["wht"][:, :].rearrange("(hc h) a -> h hc a", h=128)
        )
        hid = const.tile([128, NHC, NB], F32)      # hidden.T, hid-chunked
        nc.scalar.dma_start(
            out=hid, in_=d["hiddent"][:, :].rearrange("(hc h) b -> h hc b", h=128)
        )
        wv = const.tile([128, NAC], F32R)          # Wv as [p, ac]
        nc.scalar.dma_start(
            out=wv, in_=d["wv"][:].rearrange("(ac p) -> p ac", p=128).bitcast(F32R)
        )
        bhv = const.tile([128, NAC], F32)
        nc.scalar.dma_start(out=bhv, in_=d["bh"][:].rearrange("(ac p) -> p ac", p=128))
        becv = const.tile([128, NAC], F32)
        nc.scalar.dma_start(out=becv, in_=d["bec"][:].rearrange("(ac p) -> p ac", p=128))

        # ---- Wec.T (host-padded to 768 rows): 6 uniform K=128 chunks ----
        wect = const.tile([128, NCC, A], F32R)
        nc.sync.dma_start(
            out=wect,
            in_=d["wect"][:, :].rearrange("(cc c) a -> c cc a", c=128).bitcast(F32R),
        )

        # ---- per-(b, cc) single-writer feature tiles; b0 split across both
        #      DMA queues so the first matmul group's data lands early ----
        cnn_t = [[const.tile([128, NPIX], F32R, tag=f"cnn{b}_{cc}",
                             name=f"cnn{b}_{cc}") for cc in range(NCC)]
                 for b in range(NB)]
        im2_t = [const.tile([128, NPIX], F32R, tag=f"im2_{b}", name=f"im2_{b}")
                 for b in range(NB)]

        def load_batch_data(b):
            nc.sync.dma_start(out=im2_t[b], in_=d["im2col"][b, :, :].bitcast(F32R))
            for cc in range(NCC):
                nc.sync.dma_start(
                    out=cnn_t[b][cc],
                    in_=d["cnn"][b, cc * 128:(cc + 1) * 128, :].bitcast(F32R),
                )

        load_batch_data(0)

        # ---- prep compute: fused conv weight + per-batch bias vectors ----
        # WfusedT[t, a] = sum_k Wac_flat[k, t] * WawT[k, a]; rows 121..127
        # are zero because the padded wacf columns are zero.
        wf_ps = apsum.tile([128, A], F32)
        for kc in range(4):
            nc.tensor.matmul(
                wf_ps, lhsT=wacf[:, kc, :], rhs=wawt[:, kc, :],
                start=(kc == 0), stop=(kc == 3),
            )
        wft = const.tile([128, A], F32R)
        nc.vector.tensor_copy(wft, wf_ps)

        # bias bb[p, ac] = bh + bec ; qb[p, ac, b] = (hidden @ Wh.T)[b, a] + bb
        bb = const.tile([128, NAC], F32)
        nc.vector.tensor_add(bb, bhv, becv)
        qb = const.tile([128, NAC, NB], F32)
        for ac in range(NAC):
            q_ps = apsum.tile([128, NB], F32, tag="qps")
            for hc in range(NHC):
                nc.tensor.matmul(
                    q_ps, lhsT=wht[:, hc, ac * 128:(ac + 1) * 128],
                    rhs=hid[:, hc, :], start=(hc == 0), stop=(hc == NHC - 1),
                )
            nc.vector.tensor_scalar_add(qb[:, ac, :], q_ps, bb[:, ac:ac + 1])

        # ---- bulk loads for remaining batch items (sync queue) ----
        for b in range(1, NB):
            load_batch_data(b)

        # ---- main loop ----
        for b in range(NB):
            mask_row = maskp.tile([1, NPIX], F32, tag="mask")
            nc.gpsimd.dma_start(out=mask_row, in_=d["mask"][b, :])
            asum_row = asump.tile([1, NPIX], F32, tag="asum_in")
            nc.gpsimd.dma_start(out=asum_row, in_=d["asum"][b, :])
            em_row = rows2.tile([1, NPIX], F32, tag="em")
            dens = rows1.tile([1, 2], F32, tag="dens")
            ctxc = cpool.tile([128, NCC, 2], F32, tag="ctxc")
            for h in range(2):
                hsl = slice(h * 512, (h + 1) * 512)
                e_ps = epool.tile([1, 512], F32, tag="eps")
                for ac in range(NAC):
                    asl = slice(ac * 128, (ac + 1) * 128)
                    ps = mpool.tile([128, 512], F32, tag="mps")
                    for cc in range(NCC):
                        nc.tensor.matmul(
                            ps, lhsT=wect[:, cc, asl],
                            rhs=cnn_t[b][cc][:, hsl],
                            start=(cc == 0), stop=False,
                        )
                    nc.tensor.matmul(
                        ps, lhsT=wft[:, asl], rhs=im2_t[b][:, hsl],
                        start=False, stop=True,
                    )
                    sc = spool.tile([128, 512], F32R, tag="sc")
                    nc.scalar.activation(
                        sc, ps, AF.Tanh, bias=qb[:, ac, b:b + 1], scale=1.0
                    )
                    nc.tensor.matmul(
                        e_ps, lhsT=wv[:, ac:ac + 1], rhs=sc,
                        start=(ac == 0), stop=(ac == NAC - 1),
                    )
                # energy -> exp (no max-subtract needed; |energy| is O(1))
                exp_h = rows2.tile([1, 512], F32, tag="exph")
                nc.scalar.activation(exp_h, e_ps, AF.Exp)
                # masked exp + this half's denominator (fused)
                nc.vector.scalar_tensor_tensor(
                    out=em_row[:, hsl], in0=exp_h, scalar=1.0,
                    in1=mask_row[:, hsl], op0=MULT, op1=MULT,
                    accum_out=dens[:, h:h + 1],
                )
                # unnormalized context for this half (overlaps next half's PE)
                ab = bpool.tile([128, 512], F32, tag="ab")
                nc.gpsimd.partition_broadcast(ab, em_row[:, hsl])
                trash = tpool.tile([128, 512], F32, tag="trash")
                for cc in range(NCC):
                    nc.vector.scalar_tensor_tensor(
                        out=trash, in0=cnn_t[b][cc][:, hsl].bitcast(F32),
                        scalar=1.0, in1=ab, op0=MULT, op1=MULT,
                        accum_out=ctxc[:, cc, h:h + 1],
                    )

            # denominator and 1/(den + 1e-10)
            den = rows1.tile([1, 1], F32, tag="den")
            nc.vector.tensor_scalar_add(den, dens[:, 0:1], dens[:, 1:2])
            nc.vector.tensor_scalar_add(den, den, 1e-10)
            rcp = rows1.tile([1, 1], F32, tag="rcp")
            nc.vector.reciprocal(rcp, den)

            # alpha = em * rcp ; alpha_sum_new = alpha + alpha_sum (in place)
            alpha_row = rows2.tile([1, NPIX], F32, tag="alpha")
            nc.vector.tensor_scalar_mul(alpha_row, em_row, rcp)
            nc.scalar.dma_start(out=d["alpha"][b, :], in_=alpha_row)
            nc.vector.tensor_add(asum_row, alpha_row, asum_row)
            nc.scalar.dma_start(out=d["asum_new"][b, :], in_=asum_row)

            # context: combine halves, scale by rcp (broadcast to partitions)
            rcp128 = rows1.tile([128, 1], F32, tag="rcp128")
            nc.gpsimd.partition_broadcast(rcp128, rcp)
            ctx_fin = cpool.tile([128, NCC], F32, tag="ctxfin")
            nc.vector.tensor_add(ctx_fin, ctxc[:, :, 0], ctxc[:, :, 1])
            nc.vector.tensor_scalar_mul(ctx_fin, ctx_fin, rcp128)
            nc.scalar.dma_start(
                out=d["ctx"][b, 0:640].rearrange("(cc p) -> p cc", p=128),
                in_=ctx_fin[:, 0:5],
            )
            nc.scalar.dma_start(out=d["ctx"][b, 640:C], in_=ctx_fin[0:CREM, 5:6])


def _build():
    nc = bacc.Bacc(
        "TRN2", target_bir_lowering=False, debug=False, enable_asserts=False
    )
    d = {}
    def inp(name, shape):
        d[name] = nc.dram_tensor(name, list(shape), F32, kind="ExternalInput")[:]
    def outp(name, shape):
        d[name] = nc.dram_tensor(name, list(shape), F32, kind="ExternalOutput")[:]

    inp("cnn", (NB, 768, NPIX))
    inp("im2col", (NB, 128, NPIX))
    inp("hiddent", (HID, NB))
    inp("mask", (NB, NPIX))
    inp("asum", (NB, NPIX))
    inp("wect", (768, A))
    inp("wht", (HID, A))
    inp("wawt", (CK, A))
    inp("wacf", (CK, 128))
    inp("wv", (A,))
    inp("bh", (A,))
    inp("bec", (A,))
    outp("ctx", (NB, C))
    outp("alpha", (NB, NPIX))
    outp("asum_new", (NB, NPIX))

    with tile.TileContext(nc) as tc:
        _emit(nc, tc, d)
    nc.compile()
    return nc


def get_program():
    global _PROGRAM
    if _PROGRAM is None:
        _PROGRAM = _build()
    return _PROGRAM


def _prep_in_maps(cnn_features, hidden, alpha_sum, image_mask,
                  Wh, bh, Wec, bec, Wac, Waw, Wv, bv):
    f = np.float32
    cnn = np.zeros((B, 768, NPIX), dtype=f)
    cnn[:, :C, :] = np.asarray(cnn_features, dtype=f).reshape(B, C, NPIX)
    asum = np.ascontiguousarray(alpha_sum, dtype=f).reshape(B, NPIX)
    mask = np.ascontiguousarray(image_mask, dtype=f).reshape(B, NPIX)
    # host im2col: pure data rearrangement of alpha_sum (zero-padded windows),
    # K-padded from 121 to 128 rows with zeros
    pad = np.zeros((B, H + K - 1, W + K - 1), dtype=f)
    pad[:, K // 2:K // 2 + H, K // 2:K // 2 + W] = asum.reshape(B, H, W)
    win = np.lib.stride_tricks.sliding_window_view(pad, (H, W), axis=(1, 2))
    im2 = np.zeros((B, 128, NPIX), dtype=f)
    im2[:, :KK, :] = win.reshape(B, KK, NPIX)
    wect_pad = np.zeros((768, A), dtype=f)
    wect_pad[:C, :] = np.asarray(Wec, dtype=f).T
    # Wac flat, K-padded 121 -> 128 with zero columns
    wacf = np.zeros((CK, 128), dtype=f)
    wacf[:, :KK] = np.asarray(Wac, dtype=f).reshape(CK, KK)

    shared = {
        "wect": wect_pad,
        "wht": np.ascontiguousarray(np.asarray(Wh, dtype=f).T),
        "wawt": np.ascontiguousarray(np.asarray(Waw, dtype=f).T),
        "wacf": wacf,
        "wv": np.ascontiguousarray(np.asarray(Wv, dtype=f).reshape(A)),
        "bh": np.ascontiguousarray(np.asarray(bh, dtype=f)),
        "bec": np.ascontiguousarray(np.asarray(bec, dtype=f)),
    }
    hiddenT = np.ascontiguousarray(np.asarray(hidden, dtype=f).T)  # [HID, B]
    in_maps = []
    for m in range(NCORES):
        sl = slice(m * NB, (m + 1) * NB)
        in_maps.append({
            "cnn": np.ascontiguousarray(cnn[sl]),
            "im2col": np.ascontiguousarray(im2[sl]),
            "hiddent": np.ascontiguousarray(hiddenT[:, sl]),
            "mask": np.ascontiguousarray(mask[sl]),
            "asum": np.ascontiguousarray(asum[sl]),
            **shared,
        })
    return in_maps


def kernel(cnn_features, hidden, alpha_sum, image_mask,
           Wh, bh, Wec, bec, Wac, Waw, Wv, bv):
    global LAST_RESULT
    nc = get_program()
    in_maps = _prep_in_maps(cnn_features, hidden, alpha_sum, image_mask,
                            Wh, bh, Wec, bec, Wac, Waw, Wv, bv)
    res = run_bass_kernel_spmd(nc, in_maps, list(range(NCORES)))
    LAST_RESULT = res
    ctx = np.concatenate([res.results[m]["ctx"] for m in range(NCORES)], axis=0)
    alpha = np.concatenate(
        [res.results[m]["alpha"] for m in range(NCORES)], axis=0
    ).reshape(B, H, W)
    asum_new = np.concatenate(
        [res.results[m]["asum_new"] for m in range(NCORES)], axis=0
    ).reshape(B, 1, H, W)
    return (ctx.astype(np.float32), alpha.astype(np.float32),
            asum_new.astype(np.float32))


# revision 15
# speedup vs baseline: 1.1222x; 1.0047x over previous
"""Trainium2 Bass kernel for the sparse-attention module.

Reference computation (per batch item b):
    query   = hidden @ Wh.T + bh                          [A]
    ast     = conv2d(alpha_sum, Wac, 11x11, pad 5)        [CK, H, W]
    cov     = einsum('khw,ak->hwa', ast, Waw)             [H, W, A]
    cnn_t   = einsum('chw,ac->hwa', cnn, Wec) + bec       [H, W, A]
    score   = tanh(query + cov + cnn_t)                   [H, W, A]
    energy  = score @ Wv[0] + bv                          [H, W]
    alpha   = softmax-ish(energy) * mask / (sum + 1e-10)
    ctx     = einsum('hw,chw->c', alpha, cnn)             [C]

Kernel strategy:
  * Data-parallel over batch: 32 / 8 cores = 4 batch items per core. No
    collectives: the reference's global max-subtract cancels in the softmax
    up to the +1e-10 epsilon (relative effect ~1e-12), so it is dropped.
    bv likewise shifts all energies equally and cancels.
  * Conv fusion: cov = conv(alpha_sum, Wfused) with
    Wfused[a, ij] = sum_k Waw[a, k] * Wac[k, 0, i, j]  (computed on device),
    so the CK=512-channel conv + projection collapse into a single 121-tap
    conv, evaluated as an im2col matmul (im2col built host-side - pure
    data movement, zero-padded to K=128).
  * Main matmuls in float32r (TF32-ish, 1 cycle/row on PE); all
    contractions padded to K=128 (sub-128-K matmuls measured ~1.75x
    slower on HW).
  * Score tiles live as [A-chunk=128 partitions, 512 pixels]; tanh on
    ScalarE folds the per-(batch, A-chunk) bias (query + bh + bec).
  * energy = Wv . score via PE (contraction over A on partitions).
  * Per-(batch, pixel-half) pipelined epilogue: exp on ScalarE straight
    from PSUM, masked-exp + denominator via fused DVE scalar_tensor_tensor,
    GPSIMD partition_broadcast of the unnormalized masked exp, context
    accumulated per half on DVE, normalized at the end by the scalar
    1/(den+1e-10) - so the context work overlaps the next half's matmuls.
"""

import numpy as np

import concourse.bacc as bacc
import concourse.tile as tile
from concourse import mybir
from concourse.bass_utils import run_bass_kernel_spmd

# Problem shapes (hardcoded per contract)
B, C, H, W = 32, 684, 16, 64
HID, A, CK, K = 256, 512, 512, 11
NCORES = 8
NB = B // NCORES          # batch items per core = 4
NPIX = H * W              # 1024
KK = K * K                # 121
NCC = 6                   # C chunks: 5 x 128 + 44 (padded to 128)
CREM = C - 5 * 128        # 44
NAC = A // 128            # 4 A-chunks
NHC = HID // 128          # 2

F32 = mybir.dt.float32
F32R = mybir.dt.float32r
MULT = mybir.AluOpType.mult
ADD = mybir.AluOpType.add
AF = mybir.ActivationFunctionType

LAST_RESULT = None
_PROGRAM = None


def _emit(nc, tc, d):
    """Emit the SPMD per-core program. d maps names -> DRAM APs."""
    import contextlib

    with contextlib.ExitStack() as ctx:
        const = ctx.enter_context(tc.tile_pool(name="const", bufs=1))
        apsum = ctx.enter_context(tc.tile_pool(name="apsum", bufs=1, space="PSUM"))
        mpool = ctx.enter_context(tc.tile_pool(name="mpsum", bufs=5, space="PSUM"))
        epool = ctx.enter_context(tc.tile_pool(name="epsum", bufs=1, space="PSUM"))
        spool = ctx.enter_context(tc.tile_pool(name="score", bufs=5))
        rows2 = ctx.enter_context(tc.tile_pool(name="rows2", bufs=2))
        rows1 = ctx.enter_context(tc.tile_pool(name="rows1", bufs=1))
        bpool = ctx.enter_context(tc.tile_pool(name="bcast", bufs=2))
        tpool = ctx.enter_context(tc.tile_pool(name="trash", bufs=1))
        cpool = ctx.enter_context(tc.tile_pool(name="ctxc", bufs=2))
        maskp = ctx.enter_context(tc.tile_pool(name="maskp", bufs=2))
        asump = ctx.enter_context(tc.tile_pool(name="asump", bufs=2))

        # ---- weights needed by the prep matmuls go first (sync queue) ----
        wawt = const.tile([128, 4, A], F32R)       # Waw.T, k-chunked
        nc.sync.dma_start(
            out=wawt,
            in_=d["wawt"][:, :].rearrange("(kc k) a -> k kc a", k=128).bitcast(F32R),
        )
        wacf = const.tile([128, 4, 128], F32R)     # Wac flat (K-padded), k-chunked
        nc.sync.dma_start(
            out=wacf,
            in_=d["wacf"][:, :].rearrange("(kc k) t -> k kc t", k=128).bitcast(F32R),
        )

        # ---- small weights on the scalar queue (parallel with bulk) ----
        wht = const.tile([128, NHC, A], F32)       # Wh.T, hid-chunked
        nc.scalar.dma_start(
            out=wht, in_=d["wht"][:, :].rearrange("(hc h) a -> h hc a", h=128)
        )
        hid = const.tile([128, NHC, NB], F32)      # hidden.T, hid-chunked
        nc.scalar.dma_start(
            out=hid, in_=d["hiddent"][:, :].rearrange("(hc h) b -> h hc b", h=128)
        )
        wv = const.tile([128, NAC], F32R)          # Wv as [p, ac]
        nc.scalar.dma_start(
            out=wv, in_=d["wv"][:].rearrange("(ac p) -> p ac", p=128).bitcast(F32R)
        )
        bhv = const.tile([128, NAC], F32)
        nc.scalar.dma_start(out=bhv, in_=d["bh"][:].rearrange("(ac p) -> p ac", p=128))
        becv = const.tile([128, NAC], F32)
        nc.scalar.dma_start(out=becv, in_=d["bec"][:].rearrange("(ac p) -> p ac", p=128))

        # ---- Wec.T (host-padded to 768 rows): 6 uniform K=128 chunks ----
        wect = const.tile([128, NCC, A], F32R)
        nc.sync.dma_start(
            out=wect,
            in_=d["wect"][:, :].rearrange("(cc c) a -> c cc a", c=128).bitcast(F32R),
        )

        # ---- per-(b, cc) single-writer feature tiles; b0 split across both
        #      DMA queues so the first matmul group's data lands early ----
        cnn_t = [[const.tile([128, NPIX], F32R, tag=f"cnn{b}_{cc}",
                             name=f"cnn{b}_{cc}") for cc in range(NCC)]
                 for b in range(NB)]
        im2_t = [const.tile([128, NPIX], F32R, tag=f"im2_{b}", name=f"im2_{b}")
                 for b in range(NB)]

        def cnn_ap(b, cc, h):
            return cnn_t[b][cc][:, h * 512:(h + 1) * 512]

        def im2_ap(b, h):
            return im2_t[b][:, h * 512:(h + 1) * 512]

        def load_batch_data(b):
            nc.sync.dma_start(out=im2_t[b], in_=d["im2col"][b, :, :].bitcast(F32R))
            for cc in range(NCC):
                nc.sync.dma_start(
                    out=cnn_t[b][cc],
                    in_=d["cnn"][b, cc * 128:(cc + 1) * 128, :].bitcast(F32R),
                )

        load_batch_data(0)

        # ---- prep compute: fused conv weight + per-batch bias vectors ----
        # WfusedT[t, a] = sum_k Wac_flat[k, t] * WawT[k, a]; rows 121..127
        # are zero because the padded wacf columns are zero.
        wf_ps = apsum.tile([128, A], F32)
        for kc in range(4):
            nc.tensor.matmul(
                wf_ps, lhsT=wacf[:, kc, :], rhs=wawt[:, kc, :],
                start=(kc == 0), stop=(kc == 3),
            )
        wft = const.tile([128, A], F32R)
        nc.vector.tensor_copy(wft, wf_ps)

        # bias bb[p, ac] = bh + bec ; qb[p, ac, b] = (hidden @ Wh.T)[b, a] + bb
        bb = const.tile([128, NAC], F32)
        nc.vector.tensor_add(bb, bhv, becv)
        qb = const.tile([128, NAC, NB], F32)
        for ac in range(NAC):
            q_ps = apsum.tile([128, NB], F32, tag="qps")
            for hc in range(NHC):
                nc.tensor.matmul(
                    q_ps, lhsT=wht[:, hc, ac * 128:(ac + 1) * 128],
                    rhs=hid[:, hc, :], start=(hc == 0), stop=(hc == NHC - 1),
                )
            nc.vector.tensor_scalar_add(qb[:, ac, :], q_ps, bb[:, ac:ac + 1])

        # ---- bulk loads for remaining batch items (sync queue) ----
        for b in range(1, NB):
            load_batch_data(b)

        # ---- main loop ----
        for b in range(NB):
            mask_row = maskp.tile([1, NPIX], F32, tag="mask")
            nc.gpsimd.dma_start(out=mask_row, in_=d["mask"][b, :])
            asum_row = asump.tile([1, NPIX], F32, tag="asum_in")
            nc.gpsimd.dma_start(out=asum_row, in_=d["asum"][b, :])
            em_row = rows2.tile([1, NPIX], F32, tag="em")
            dens = rows1.tile([1, 2], F32, tag="dens")
            ctxc = cpool.tile([128, NCC, 2], F32, tag="ctxc")
            for h in range(2):
                hsl = slice(h * 512, (h + 1) * 512)
                e_ps = epool.tile([1, 512], F32, tag="eps")
                pss = [mpool.tile([128, 512], F32, tag="mps", name=f"ps{ac}")
                       for ac in range(NAC)]
                for cc in range(NCC):
                    for ac in range(NAC):
                        nc.tensor.matmul(
                            pss[ac], lhsT=wect[:, cc, ac * 128:(ac + 1) * 128],
                            rhs=cnn_ap(b, cc, h),
                            start=(cc == 0), stop=False,
                        )
                for ac in range(NAC):
                    nc.tensor.matmul(
                        pss[ac], lhsT=wft[:, ac * 128:(ac + 1) * 128],
                        rhs=im2_ap(b, h), start=False, stop=True,
                    )
                scs = []
                for ac in range(NAC):
                    sc = spool.tile([128, 512], F32R, tag="sc", name=f"sc{ac}")
                    nc.scalar.activation(
                        sc, pss[ac], AF.Tanh, bias=qb[:, ac, b:b + 1], scale=1.0
                    )
                    scs.append(sc)
                for ac in range(NAC):
                    nc.tensor.matmul(
                        e_ps, lhsT=wv[:, ac:ac + 1], rhs=scs[ac],
                        start=(ac == 0), stop=(ac == NAC - 1),
                    )
                # energy -> exp (no max-subtract needed; |energy| is O(1))
                exp_h = rows2.tile([1, 512], F32, tag="exph")
                nc.scalar.activation(exp_h, e_ps, AF.Exp)
                # masked exp + this half's denominator (fused)
                nc.vector.scalar_tensor_tensor(
                    out=em_row[:, hsl], in0=exp_h, scalar=1.0,
                    in1=mask_row[:, hsl], op0=MULT, op1=MULT,
                    accum_out=dens[:, h:h + 1],
                )
                # unnormalized context for this half (overlaps next half's PE)
                ab = bpool.tile([128, 512], F32, tag="ab")
                nc.gpsimd.partition_broadcast(ab, em_row[:, hsl])
                trash = tpool.tile([128, 512], F32, tag="trash")
                for cc in range(NCC):
                    nc.vector.scalar_tensor_tensor(
                        out=trash, in0=cnn_t[b][cc][:, hsl].bitcast(F32),
                        scalar=1.0, in1=ab, op0=MULT, op1=MULT,
                        accum_out=ctxc[:, cc, h:h + 1],
                    )

            # denominator and 1/(den + 1e-10)
            den = rows1.tile([1, 1], F32, tag="den")
            nc.vector.tensor_scalar_add(den, dens[:, 0:1], dens[:, 1:2])
            nc.vector.tensor_scalar_add(den, den, 1e-10)
            rcp = rows1.tile([1, 1], F32, tag="rcp")
            nc.vector.reciprocal(rcp, den)

            # alpha = em * rcp ; alpha_sum_new = alpha + alpha_sum (in place)
            alpha_row = rows2.tile([1, NPIX], F32, tag="alpha")
            nc.vector.tensor_scalar_mul(alpha_row, em_row, rcp)
            nc.scalar.dma_start(out=d["alpha"][b, :], in_=alpha_row)
            nc.vector.tensor_add(asum_row, alpha_row, asum_row)
            nc.scalar.dma_start(out=d["asum_new"][b, :], in_=asum_row)

            # context: combine halves, scale by rcp (broadcast to partitions)
            rcp128 = rows1.tile([128, 1], F32, tag="rcp128")
            nc.gpsimd.partition_broadcast(rcp128, rcp)
            ctx_fin = cpool.tile([128, NCC], F32, tag="ctxfin")
            nc.vector.tensor_add(ctx_fin, ctxc[:, :, 0], ctxc[:, :, 1])
            nc.vector.tensor_scalar_mul(ctx_fin, ctx_fin, rcp128)
            nc.scalar.dma_start(
                out=d["ctx"][b, 0:640].rearrange("(cc p) -> p cc", p=128),
                in_=ctx_fin[:, 0:5],
            )
            nc.scalar.dma_start(out=d["ctx"][b, 640:C], in_=ctx_fin[0:CREM, 5:6])


def _build():
    nc = bacc.Bacc(
        "TRN2", target_bir_lowering=False, debug=False, enable_asserts=False
    )
    d = {}
    def inp(name, shape):
        d[name] = nc.dram_tensor(name, list(shape), F32, kind="ExternalInput")[:]
    def outp(name, shape):
        d[name] = nc.dram_tensor(name, list(shape), F32, kind="ExternalOutput")[:]

    inp("cnn", (NB, 768, NPIX))
    inp("im2col", (NB, 128, NPIX))
    inp("hiddent", (HID, NB))
    inp("mask", (NB, NPIX))
    inp("asum", (NB, NPIX))
    inp("wect", (768, A))
    inp("wht", (HID, A))
    inp("wawt", (CK, A))
    inp("wacf", (CK, 128))
    inp("wv", (A,))
    inp("bh", (A,))
    inp("bec", (A,))
    outp("ctx", (NB, C))
    outp("alpha", (NB, NPIX))
    outp("asum_new", (NB, NPIX))

    with tile.TileContext(nc) as tc:
        _emit(nc, tc, d)
    nc.compile()
    return nc


def get_program():
    global _PROGRAM
    if _PROGRAM is None:
        _PROGRAM = _build()
    return _PROGRAM


def _prep_in_maps(cnn_features, hidden, alpha_sum, image_mask,
                  Wh, bh, Wec, bec, Wac, Waw, Wv, bv):
    f = np.float32
    cnn = np.zeros((B, 768, NPIX), dtype=f)
    cnn[:, :C, :] = np.asarray(cnn_features, dtype=f).reshape(B, C, NPIX)
    asum = np.ascontiguousarray(alpha_sum, dtype=f).reshape(B, NPIX)
    mask = np.ascontiguousarray(image_mask, dtype=f).reshape(B, NPIX)
    # host im2col: pure data rearrangement of alpha_sum (zero-padded windows),
    # K-padded from 121 to 128 rows with zeros
    pad = np.zeros((B, H + K - 1, W + K - 1), dtype=f)
    pad[:, K // 2:K // 2 + H, K // 2:K // 2 + W] = asum.reshape(B, H, W)
    win = np.lib.stride_tricks.sliding_window_view(pad, (H, W), axis=(1, 2))
    im2 = np.zeros((B, 128, NPIX), dtype=f)
    im2[:, :KK, :] = win.reshape(B, KK, NPIX)
    wect_pad = np.zeros((768, A), dtype=f)
    wect_pad[:C, :] = np.asarray(Wec, dtype=f).T
    # Wac flat, K-padded 121 -> 128 with zero columns
    wacf = np.zeros((CK, 128), dtype=f)
    wacf[:, :KK] = np.asarray(Wac, dtype=f).reshape(CK, KK)

    shared = {
        "wect": wect_pad,
        "wht": np.ascontiguousarray(np.asarray(Wh, dtype=f).T),
        "wawt": np.ascontiguousarray(np.asarray(Waw, dtype=f).T),
        "wacf": wacf,
        "wv": np.ascontiguousarray(np.asarray(Wv, dtype=f).reshape(A)),
        "bh": np.ascontiguousarray(np.asarray(bh, dtype=f)),
        "bec": np.ascontiguousarray(np.asarray(bec, dtype=f)),
    }
    hiddenT = np.ascontiguousarray(np.asarray(hidden, dtype=f).T)  # [HID, B]
    in_maps = []
    for m in range(NCORES):
        sl = slice(m * NB, (m + 1) * NB)
        in_maps.append({
            "cnn": np.ascontiguousarray(cnn[sl]),
            "im2col": np.ascontiguousarray(im2[sl]),
            "hiddent": np.ascontiguousarray(hiddenT[:, sl]),
            "mask": np.ascontiguousarray(mask[sl]),
            "asum": np.ascontiguousarray(asum[sl]),
            **shared,
        })
    return in_maps


def kernel(cnn_features, hidden, alpha_sum, image_mask,
           Wh, bh, Wec, bec, Wac, Waw, Wv, bv):
    global LAST_RESULT
    nc = get_program()
    in_maps = _prep_in_maps(cnn_features, hidden, alpha_sum, image_mask,
                            Wh, bh, Wec, bec, Wac, Waw, Wv, bv)
    res = run_bass_kernel_spmd(nc, in_maps, list(range(NCORES)))
    LAST_RESULT = res
    ctx = np.concatenate([res.results[m]["ctx"] for m in range(NCORES)], axis=0)
    alpha = np.concatenate(
        [res.results[m]["alpha"] for m in range(NCORES)], axis=0
    ).reshape(B, H, W)
    asum_new = np.concatenate(
        [res.results[m]["asum_new"] for m in range(NCORES)], axis=0
    ).reshape(B, 1, H, W)
    return (ctx.astype(np.float32), alpha.astype(np.float32),
            asum_new.astype(np.float32))


# revision 17
# speedup vs baseline: 1.2325x; 1.0983x over previous
"""Trainium2 Bass kernel for the sparse-attention module.

Reference computation (per batch item b):
    query   = hidden @ Wh.T + bh                          [A]
    ast     = conv2d(alpha_sum, Wac, 11x11, pad 5)        [CK, H, W]
    cov     = einsum('khw,ak->hwa', ast, Waw)             [H, W, A]
    cnn_t   = einsum('chw,ac->hwa', cnn, Wec) + bec       [H, W, A]
    score   = tanh(query + cov + cnn_t)                   [H, W, A]
    energy  = score @ Wv[0] + bv                          [H, W]
    alpha   = softmax-ish(energy) * mask / (sum + 1e-10)
    ctx     = einsum('hw,chw->c', alpha, cnn)             [C]

Kernel strategy:
  * Data-parallel over batch: 32 / 8 cores = 4 batch items per core. No
    collectives: the reference's global max-subtract cancels in the softmax
    up to the +1e-10 epsilon (relative effect ~1e-12), so it is dropped.
    bv likewise shifts all energies equally and cancels.
  * Conv fusion: cov = conv(alpha_sum, Wfused) with
    Wfused[a, ij] = sum_k Waw[a, k] * Wac[k, 0, i, j]  (computed on device),
    so the CK=512-channel conv + projection collapse into a single 121-tap
    conv, evaluated as an im2col matmul (im2col built host-side - pure
    data movement, zero-padded to K=128).
  * Main matmuls in float32r (TF32-ish, 1 cycle/row on PE); all
    contractions padded to K=128 (sub-128-K matmuls measured ~1.75x
    slower on HW).
  * Score tiles live as [A-chunk=128 partitions, 512 pixels]; tanh on
    ScalarE folds the per-(batch, A-chunk) bias (query + bh + bec).
  * energy = Wv . score via PE (contraction over A on partitions).
  * Per-(batch, pixel-half) pipelined epilogue: exp on ScalarE straight
    from PSUM, masked-exp + denominator via fused DVE scalar_tensor_tensor,
    GPSIMD partition_broadcast of the unnormalized masked exp, context
    accumulated per half on DVE, normalized at the end by the scalar
    1/(den+1e-10) - so the context work overlaps the next half's matmuls.
"""

import numpy as np

import concourse.bacc as bacc
import concourse.tile as tile
from concourse import mybir
from concourse.bass_utils import run_bass_kernel_spmd

# Problem shapes (hardcoded per contract)
B, C, H, W = 32, 684, 16, 64
HID, A, CK, K = 256, 512, 512, 11
NCORES = 8
NB = B // NCORES          # batch items per core = 4
NPIX = H * W              # 1024
KK = K * K                # 121
NCC = 6                   # C chunks: 5 x 128 + 44 (padded to 128)
CREM = C - 5 * 128        # 44
NAC = A // 128            # 4 A-chunks
NHC = HID // 128          # 2

F32 = mybir.dt.float32
F32R = mybir.dt.float32r
MULT = mybir.AluOpType.mult
ADD = mybir.AluOpType.add
AF = mybir.ActivationFunctionType

LAST_RESULT = None
_PROGRAM = None


def _emit(nc, tc, d):
    """Emit the SPMD per-core program. d maps names -> DRAM APs."""
    import contextlib

    with contextlib.ExitStack() as ctx:
        const = ctx.enter_context(tc.tile_pool(name="const", bufs=1))
        apsum = ctx.enter_context(tc.tile_pool(name="apsum", bufs=1, space="PSUM"))
        mpool = ctx.enter_context(tc.tile_pool(name="mpsum", bufs=5, space="PSUM"))
        epool = ctx.enter_context(tc.tile_pool(name="epsum", bufs=1, space="PSUM"))
        spool = ctx.enter_context(tc.tile_pool(name="score", bufs=5))
        rows2 = ctx.enter_context(tc.tile_pool(name="rows2", bufs=2))
        rows1 = ctx.enter_context(tc.tile_pool(name="rows1", bufs=1))
        bpool = ctx.enter_context(tc.tile_pool(name="bcast", bufs=2))
        tpool = ctx.enter_context(tc.tile_pool(name="trash", bufs=1))
        cpool = ctx.enter_context(tc.tile_pool(name="ctxc", bufs=2))
        maskp = ctx.enter_context(tc.tile_pool(name="maskp", bufs=2))
        asump = ctx.enter_context(tc.tile_pool(name="asump", bufs=2))

        # ---- weights needed by the prep matmuls go first (sync queue) ----
        wawt = const.tile([128, 4, A], F32R)       # Waw.T, k-chunked
        nc.sync.dma_start(
            out=wawt,
            in_=d["wawt"][:, :].rearrange("(kc k) a -> k kc a", k=128).bitcast(F32R),
        )
        wacf = const.tile([128, 4, 128], F32R)     # Wac flat (K-padded), k-chunked
        nc.sync.dma_start(
            out=wacf,
            in_=d["wacf"][:, :].rearrange("(kc k) t -> k kc t", k=128).bitcast(F32R),
        )

        # ---- small weights on the scalar queue (parallel with bulk) ----
        wht = const.tile([128, NHC, A], F32)       # Wh.T, hid-chunked
        nc.scalar.dma_start(
            out=wht, in_=d["wht"][:, :].rearrange("(hc h) a -> h hc a", h=128)
        )
        hid = const.tile([128, NHC, NB], F32)      # hidden.T, hid-chunked
        nc.scalar.dma_start(
            out=hid, in_=d["hiddent"][:, :].rearrange("(hc h) b -> h hc b", h=128)
        )
        wv = const.tile([128, NAC], F32R)          # Wv as [p, ac]
        nc.scalar.dma_start(
            out=wv, in_=d["wv"][:].rearrange("(ac p) -> p ac", p=128).bitcast(F32R)
        )
        bhv = const.tile([128, NAC], F32)
        nc.scalar.dma_start(out=bhv, in_=d["bh"][:].rearrange("(ac p) -> p ac", p=128))
        becv = const.tile([128, NAC], F32)
        nc.scalar.dma_start(out=becv, in_=d["bec"][:].rearrange("(ac p) -> p ac", p=128))

        # ---- Wec.T (host-padded to 768 rows), split per A-chunk so the
        #      first matmul group only waits on its own slice ----
        wect_ac = [const.tile([128, NCC, 128], F32R, tag=f"wect{ac}",
                              name=f"wect{ac}") for ac in range(NAC)]

        def load_wect(ac):
            nc.sync.dma_start(
                out=wect_ac[ac],
                in_=d["wect"][:, ac * 128:(ac + 1) * 128]
                .rearrange("(cc c) a -> c cc a", c=128).bitcast(F32R),
            )

        load_wect(0)

        # ---- per-(b, cc) single-writer feature tiles; b0 split across both
        #      DMA queues so the first matmul group's data lands early ----
        cnn_b0 = [[const.tile([128, 512], F32R, tag=f"cnn0_{cc}_{h}",
                              name=f"cnn0_{cc}_{h}") for h in range(2)]
                  for cc in range(NCC)]
        im2_b0 = [const.tile([128, 512], F32R, tag=f"im20_{h}", name=f"im20_{h}")
                  for h in range(2)]
        cnn_t = [None] + [[const.tile([128, NPIX], F32R, tag=f"cnn{b}_{cc}",
                                      name=f"cnn{b}_{cc}") for cc in range(NCC)]
                          for b in range(1, NB)]
        im2_t = [None] + [const.tile([128, NPIX], F32R, tag=f"im2_{b}",
                                     name=f"im2_{b}") for b in range(1, NB)]

        def load_b0_half(h):
            hs = slice(h * 512, (h + 1) * 512)
            nc.sync.dma_start(out=im2_b0[h], in_=d["im2col"][0, :, hs].bitcast(F32R))
            for cc in range(NCC):
                nc.sync.dma_start(
                    out=cnn_b0[cc][h],
                    in_=d["cnn"][0, cc * 128:(cc + 1) * 128, hs].bitcast(F32R),
                )

        def cnn_ap(b, cc, h):
            if b == 0:
                return cnn_b0[cc][h]
            return cnn_t[b][cc][:, h * 512:(h + 1) * 512]

        def im2_ap(b, h):
            if b == 0:
                return im2_b0[h]
            return im2_t[b][:, h * 512:(h + 1) * 512]

        def load_batch_data(b):
            nc.sync.dma_start(out=im2_t[b], in_=d["im2col"][b, :, :].bitcast(F32R))
            for cc in range(NCC):
                nc.sync.dma_start(
                    out=cnn_t[b][cc],
                    in_=d["cnn"][b, cc * 128:(cc + 1) * 128, :].bitcast(F32R),
                )

        load_b0_half(0)
        for _ac in range(1, NAC):
            load_wect(_ac)
        load_b0_half(1)

        # ---- prep compute: fused conv weight + per-batch bias vectors ----
        # WfusedT[t, a] = sum_k Wac_flat[k, t] * WawT[k, a]; rows 121..127
        # are zero because the padded wacf columns are zero.
        wf_ps = apsum.tile([128, A], F32)
        for kc in range(4):
            nc.tensor.matmul(
                wf_ps, lhsT=wacf[:, kc, :], rhs=wawt[:, kc, :],
                start=(kc == 0), stop=(kc == 3),
            )
        wft = const.tile([128, A], F32R)
        nc.vector.tensor_copy(wft, wf_ps)

        # bias bb[p, ac] = bh + bec ; qb[p, ac, b] = (hidden @ Wh.T)[b, a] + bb
        bb = const.tile([128, NAC], F32)
        nc.vector.tensor_add(bb, bhv, becv)
        qb = const.tile([128, NAC, NB], F32)
        for ac in range(NAC):
            q_ps = apsum.tile([128, NB], F32, tag="qps")
            for hc in range(NHC):
                nc.tensor.matmul(
                    q_ps, lhsT=wht[:, hc, ac * 128:(ac + 1) * 128],
                    rhs=hid[:, hc, :], start=(hc == 0), stop=(hc == NHC - 1),
                )
            nc.vector.tensor_scalar_add(qb[:, ac, :], q_ps, bb[:, ac:ac + 1])

        # ---- bulk loads for remaining batch items (sync queue) ----
        for b in range(1, NB):
            load_batch_data(b)

        # ---- main loop ----
        for b in range(NB):
            mask_row = maskp.tile([1, NPIX], F32, tag="mask")
            nc.gpsimd.dma_start(out=mask_row, in_=d["mask"][b, :])
            asum_row = asump.tile([1, NPIX], F32, tag="asum_in")
            nc.gpsimd.dma_start(out=asum_row, in_=d["asum"][b, :])
            em_row = rows2.tile([1, NPIX], F32, tag="em")
            dens = rows1.tile([1, 2], F32, tag="dens")
            ctxc = cpool.tile([128, NCC, 2], F32, tag="ctxc")
            for h in range(2):
                hsl = slice(h * 512, (h + 1) * 512)
                e_ps = epool.tile([1, 512], F32, tag="eps")
                pss = [mpool.tile([128, 512], F32, tag="mps", name=f"ps{ac}")
                       for ac in range(NAC)]
                for cc in range(NCC):
                    for ac in range(NAC):
                        nc.tensor.matmul(
                            pss[ac], lhsT=wect_ac[ac][:, cc, :],
                            rhs=cnn_ap(b, cc, h),
                            start=(cc == 0), stop=False,
                        )
                for ac in range(NAC):
                    nc.tensor.matmul(
                        pss[ac], lhsT=wft[:, ac * 128:(ac + 1) * 128],
                        rhs=im2_ap(b, h), start=False, stop=True,
                    )
                scs = []
                for ac in range(NAC):
                    sc = spool.tile([128, 512], F32R, tag="sc", name=f"sc{ac}")
                    nc.scalar.activation(
                        sc, pss[ac], AF.Tanh, bias=qb[:, ac, b:b + 1], scale=1.0
                    )
                    scs.append(sc)
                for ac in range(NAC):
                    nc.tensor.matmul(
                        e_ps, lhsT=wv[:, ac:ac + 1], rhs=scs[ac],
                        start=(ac == 0), stop=(ac == NAC - 1),
                    )
                # energy -> exp (no max-subtract needed; |energy| is O(1))
                exp_h = rows2.tile([1, 512], F32, tag="exph")
                nc.scalar.activation(exp_h, e_ps, AF.Exp)
                # masked exp + this half's denominator (fused)
                nc.vector.scalar_tensor_tensor(
                    out=em_row[:, hsl], in0=exp_h, scalar=1.0,
                    in1=mask_row[:, hsl], op0=MULT, op1=MULT,
                    accum_out=dens[:, h:h + 1],
                )
                # unnormalized context for this half (overlaps next half's PE)
                ab = bpool.tile([128, 512], F32, tag="ab")
                nc.gpsimd.partition_broadcast(ab, em_row[:, hsl])
                trash = tpool.tile([128, 512], F32, tag="trash")
                for cc in range(NCC):
                    nc.vector.scalar_tensor_tensor(
                        out=trash, in0=cnn_ap(b, cc, h).bitcast(F32),
                        scalar=1.0, in1=ab, op0=MULT, op1=MULT,
                        accum_out=ctxc[:, cc, h:h + 1],
                    )

            # denominator and 1/(den + 1e-10)
            den = rows1.tile([1, 1], F32, tag="den")
            nc.vector.tensor_scalar_add(den, dens[:, 0:1], dens[:, 1:2])
            nc.vector.tensor_scalar_add(den, den, 1e-10)
            rcp = rows1.tile([1, 1], F32, tag="rcp")
            nc.vector.reciprocal(rcp, den)

            # alpha = em * rcp ; alpha_sum_new = alpha + alpha_sum (in place)
            alpha_row = rows2.tile([1, NPIX], F32, tag="alpha")
            nc.vector.tensor_scalar_mul(alpha_row, em_row, rcp)
            nc.sync.dma_start(out=d["alpha"][b, :], in_=alpha_row)
            nc.vector.tensor_add(asum_row, alpha_row, asum_row)
            nc.sync.dma_start(out=d["asum_new"][b, :], in_=asum_row)

            # context: combine halves, scale by rcp (broadcast to partitions)
            rcp128 = rows1.tile([128, 1], F32, tag="rcp128")
            nc.gpsimd.partition_broadcast(rcp128, rcp)
            ctx_fin = cpool.tile([128, NCC], F32, tag="ctxfin")
            nc.vector.tensor_add(ctx_fin, ctxc[:, :, 0], ctxc[:, :, 1])
            nc.vector.tensor_scalar_mul(ctx_fin, ctx_fin, rcp128)
            nc.sync.dma_start(
                out=d["ctx"][b, 0:640].rearrange("(cc p) -> p cc", p=128),
                in_=ctx_fin[:, 0:5],
            )
            nc.sync.dma_start(out=d["ctx"][b, 640:C], in_=ctx_fin[0:CREM, 5:6])


def _build():
    nc = bacc.Bacc(
        "TRN2", target_bir_lowering=False, debug=False, enable_asserts=False
    )
    d = {}
    def inp(name, shape):
        d[name] = nc.dram_tensor(name, list(shape), F32, kind="ExternalInput")[:]
    def outp(name, shape):
        d[name] = nc.dram_tensor(name, list(shape), F32, kind="ExternalOutput")[:]

    inp("cnn", (NB, 768, NPIX))
    inp("im2col", (NB, 128, NPIX))
    inp("hiddent", (HID, NB))
    inp("mask", (NB, NPIX))
    inp("asum", (NB, NPIX))
    inp("wect", (768, A))
    inp("wht", (HID, A))
    inp("wawt", (CK, A))
    inp("wacf", (CK, 128))
    inp("wv", (A,))
    inp("bh", (A,))
    inp("bec", (A,))
    outp("ctx", (NB, C))
    outp("alpha", (NB, NPIX))
    outp("asum_new", (NB, NPIX))

    with tile.TileContext(nc) as tc:
        _emit(nc, tc, d)
    nc.compile()
    return nc


def get_program():
    global _PROGRAM
    if _PROGRAM is None:
        _PROGRAM = _build()
    return _PROGRAM


def _prep_in_maps(cnn_features, hidden, alpha_sum, image_mask,
                  Wh, bh, Wec, bec, Wac, Waw, Wv, bv):
    f = np.float32
    cnn = np.zeros((B, 768, NPIX), dtype=f)
    cnn[:, :C, :] = np.asarray(cnn_features, dtype=f).reshape(B, C, NPIX)
    asum = np.ascontiguousarray(alpha_sum, dtype=f).reshape(B, NPIX)
    mask = np.ascontiguousarray(image_mask, dtype=f).reshape(B, NPIX)
    # host im2col: pure data rearrangement of alpha_sum (zero-padded windows),
    # K-padded from 121 to 128 rows with zeros
    pad = np.zeros((B, H + K - 1, W + K - 1), dtype=f)
    pad[:, K // 2:K // 2 + H, K // 2:K // 2 + W] = asum.reshape(B, H, W)
    win = np.lib.stride_tricks.sliding_window_view(pad, (H, W), axis=(1, 2))
    im2 = np.zeros((B, 128, NPIX), dtype=f)
    im2[:, :KK, :] = win.reshape(B, KK, NPIX)
    wect_pad = np.zeros((768, A), dtype=f)
    wect_pad[:C, :] = np.asarray(Wec, dtype=f).T
    # Wac flat, K-padded 121 -> 128 with zero columns
    wacf = np.zeros((CK, 128), dtype=f)
    wacf[:, :KK] = np.asarray(Wac, dtype=f).reshape(CK, KK)

    shared = {
        "wect": wect_pad,
        "wht": np.ascontiguousarray(np.asarray(Wh, dtype=f).T),
        "wawt": np.ascontiguousarray(np.asarray(Waw, dtype=f).T),
        "wacf": wacf,
        "wv": np.ascontiguousarray(np.asarray(Wv, dtype=f).reshape(A)),
        "bh": np.ascontiguousarray(np.asarray(bh, dtype=f)),
        "bec": np.ascontiguousarray(np.asarray(bec, dtype=f)),
    }
    hiddenT = np.ascontiguousarray(np.asarray(hidden, dtype=f).T)  # [HID, B]
    in_maps = []
    for m in range(NCORES):
        sl = slice(m * NB, (m + 1) * NB)
        in_maps.append({
            "cnn": np.ascontiguousarray(cnn[sl]),
            "im2col": np.ascontiguousarray(im2[sl]),
            "hiddent": np.ascontiguousarray(hiddenT[:, sl]),
            "mask": np.ascontiguousarray(mask[sl]),
            "asum": np.ascontiguousarray(asum[sl]),
            **shared,
        })
    return in_maps


def kernel(cnn_features, hidden, alpha_sum, image_mask,
           Wh, bh, Wec, bec, Wac, Waw, Wv, bv):
    global LAST_RESULT
    nc = get_program()
    in_maps = _prep_in_maps(cnn_features, hidden, alpha_sum, image_mask,
                            Wh, bh, Wec, bec, Wac, Waw, Wv, bv)
    res = run_bass_kernel_spmd(nc, in_maps, list(range(NCORES)))
    LAST_RESULT = res
    ctx = np.concatenate([res.results[m]["ctx"] for m in range(NCORES)], axis=0)
    alpha = np.concatenate(
        [res.results[m]["alpha"] for m in range(NCORES)], axis=0
    ).reshape(B, H, W)
    asum_new = np.concatenate(
        [res.results[m]["asum_new"] for m in range(NCORES)], axis=0
    ).reshape(B, 1, H, W)
    return (ctx.astype(np.float32), alpha.astype(np.float32),
            asum_new.astype(np.float32))


# revision 18
# speedup vs baseline: 1.2562x; 1.0192x over previous
"""Trainium2 Bass kernel for the sparse-attention module.

Reference computation (per batch item b):
    query   = hidden @ Wh.T + bh                          [A]
    ast     = conv2d(alpha_sum, Wac, 11x11, pad 5)        [CK, H, W]
    cov     = einsum('khw,ak->hwa', ast, Waw)             [H, W, A]
    cnn_t   = einsum('chw,ac->hwa', cnn, Wec) + bec       [H, W, A]
    score   = tanh(query + cov + cnn_t)                   [H, W, A]
    energy  = score @ Wv[0] + bv                          [H, W]
    alpha   = softmax-ish(energy) * mask / (sum + 1e-10)
    ctx     = einsum('hw,chw->c', alpha, cnn)             [C]

Kernel strategy:
  * Data-parallel over batch: 32 / 8 cores = 4 batch items per core. No
    collectives: the reference's global max-subtract cancels in the softmax
    up to the +1e-10 epsilon (relative effect ~1e-12), so it is dropped.
    bv likewise shifts all energies equally and cancels.
  * Conv fusion: cov = conv(alpha_sum, Wfused) with
    Wfused[a, ij] = sum_k Waw[a, k] * Wac[k, 0, i, j]  (computed on device),
    so the CK=512-channel conv + projection collapse into a single 121-tap
    conv, evaluated as an im2col matmul (im2col built host-side - pure
    data movement, zero-padded to K=128).
  * Main matmuls in float32r (TF32-ish, 1 cycle/row on PE); all
    contractions padded to K=128 (sub-128-K matmuls measured ~1.75x
    slower on HW).
  * Score tiles live as [A-chunk=128 partitions, 512 pixels]; tanh on
    ScalarE folds the per-(batch, A-chunk) bias (query + bh + bec).
  * energy = Wv . score via PE (contraction over A on partitions).
  * Per-(batch, pixel-half) pipelined epilogue: exp on ScalarE straight
    from PSUM, masked-exp + denominator via fused DVE scalar_tensor_tensor,
    GPSIMD partition_broadcast of the unnormalized masked exp, context
    accumulated per half on DVE, normalized at the end by the scalar
    1/(den+1e-10) - so the context work overlaps the next half's matmuls.
  * DMA scheduling: single-writer tiles only (multi-writer tiles serialize
    consumers on the last writer); batch 0 split per (C-chunk, pixel-half)
    and Wec.T per A-chunk so the first matmul group needs only ~3.5 MB;
    bulk loads on the sync HWDGE queue, small weights on the scalar queue,
    per-batch mask/alpha_sum rows on the GPSIMD SWDGE queue, outputs back
    on sync (idle by then) - keeps DMA issue ops off the ScalarE row,
    which carries latency-critical tanh/exp.
"""

import numpy as np

import concourse.bacc as bacc
import concourse.tile as tile
from concourse import mybir
from concourse.bass_utils import run_bass_kernel_spmd

# Problem shapes (hardcoded per contract)
B, C, H, W = 32, 684, 16, 64
HID, A, CK, K = 256, 512, 512, 11
NCORES = 8
NB = B // NCORES          # batch items per core = 4
NPIX = H * W              # 1024
KK = K * K                # 121
NCC = 6                   # C chunks: 5 x 128 + 44 (padded to 128)
CREM = C - 5 * 128        # 44
NAC = A // 128            # 4 A-chunks
NHC = HID // 128          # 2

F32 = mybir.dt.float32
F32R = mybir.dt.float32r
MULT = mybir.AluOpType.mult
ADD = mybir.AluOpType.add
AF = mybir.ActivationFunctionType

LAST_RESULT = None
_PROGRAM = None


def _emit(nc, tc, d):
    """Emit the SPMD per-core program. d maps names -> DRAM APs."""
    import contextlib

    with contextlib.ExitStack() as ctx:
        const = ctx.enter_context(tc.tile_pool(name="const", bufs=1))
        apsum = ctx.enter_context(tc.tile_pool(name="apsum", bufs=1, space="PSUM"))
        mpool = ctx.enter_context(tc.tile_pool(name="mpsum", bufs=5, space="PSUM"))
        epool = ctx.enter_context(tc.tile_pool(name="epsum", bufs=1, space="PSUM"))
        spool = ctx.enter_context(tc.tile_pool(name="score", bufs=5))
        rows2 = ctx.enter_context(tc.tile_pool(name="rows2", bufs=2))
        rows1 = ctx.enter_context(tc.tile_pool(name="rows1", bufs=1))
        bpool = ctx.enter_context(tc.tile_pool(name="bcast", bufs=2))
        tpool = ctx.enter_context(tc.tile_pool(name="trash", bufs=1))
        cpool = ctx.enter_context(tc.tile_pool(name="ctxc", bufs=2))
        maskp = ctx.enter_context(tc.tile_pool(name="maskp", bufs=2))
        asump = ctx.enter_context(tc.tile_pool(name="asump", bufs=2))

        # ---- weights needed by the prep matmuls go first (sync queue) ----
        wawt = const.tile([128, 4, A], F32R)       # Waw.T, k-chunked
        nc.sync.dma_start(
            out=wawt,
            in_=d["wawt"][:, :].rearrange("(kc k) a -> k kc a", k=128).bitcast(F32R),
        )
        wacf = const.tile([128, 4, 128], F32R)     # Wac flat (K-padded), k-chunked
        nc.sync.dma_start(
            out=wacf,
            in_=d["wacf"][:, :].rearrange("(kc k) t -> k kc t", k=128).bitcast(F32R),
        )

        # ---- small weights on the scalar queue (parallel with bulk) ----
        wht = const.tile([128, NHC, A], F32)       # Wh.T, hid-chunked
        nc.scalar.dma_start(
            out=wht, in_=d["wht"][:, :].rearrange("(hc h) a -> h hc a", h=128)
        )
        hid = const.tile([128, NHC, NB], F32)      # hidden.T, hid-chunked
        nc.scalar.dma_start(
            out=hid, in_=d["hiddent"][:, :].rearrange("(hc h) b -> h hc b", h=128)
        )
        wv = const.tile([128, NAC], F32R)          # Wv as [p, ac]
        nc.scalar.dma_start(
            out=wv, in_=d["wv"][:].rearrange("(ac p) -> p ac", p=128).bitcast(F32R)
        )
        bhv = const.tile([128, NAC], F32)
        nc.scalar.dma_start(out=bhv, in_=d["bh"][:].rearrange("(ac p) -> p ac", p=128))
        becv = const.tile([128, NAC], F32)
        nc.scalar.dma_start(out=becv, in_=d["bec"][:].rearrange("(ac p) -> p ac", p=128))

        # ---- Wec.T (host-padded to 768 rows), split per A-chunk so the
        #      first matmul group only waits on its own slice ----
        wect_ac = [const.tile([128, NCC, 128], F32R, tag=f"wect{ac}",
                              name=f"wect{ac}") for ac in range(NAC)]

        def load_wect(ac):
            nc.sync.dma_start(
                out=wect_ac[ac],
                in_=d["wect"][:, ac * 128:(ac + 1) * 128]
                .rearrange("(cc c) a -> c cc a", c=128).bitcast(F32R),
            )

        load_wect(0)

        # ---- per-(b, cc) single-writer feature tiles; b0 split across both
        #      DMA queues so the first matmul group's data lands early ----
        cnn_b0 = [[const.tile([128, 512], F32R, tag=f"cnn0_{cc}_{h}",
                              name=f"cnn0_{cc}_{h}") for h in range(2)]
                  for cc in range(NCC)]
        im2_b0 = [const.tile([128, 512], F32R, tag=f"im20_{h}", name=f"im20_{h}")
                  for h in range(2)]
        cnn_t = [None] + [[const.tile([128, NPIX], F32R, tag=f"cnn{b}_{cc}",
                                      name=f"cnn{b}_{cc}") for cc in range(NCC)]
                          for b in range(1, NB)]
        im2_t = [None] + [const.tile([128, NPIX], F32R, tag=f"im2_{b}",
                                     name=f"im2_{b}") for b in range(1, NB)]

        def load_b0_half(h):
            hs = slice(h * 512, (h + 1) * 512)
            nc.sync.dma_start(out=im2_b0[h], in_=d["im2col"][0, :, hs].bitcast(F32R))
            for cc in range(NCC):
                nc.sync.dma_start(
                    out=cnn_b0[cc][h],
                    in_=d["cnn"][0, cc * 128:(cc + 1) * 128, hs].bitcast(F32R),
                )

        def cnn_ap(b, cc, h):
            if b == 0:
                return cnn_b0[cc][h]
            return cnn_t[b][cc][:, h * 512:(h + 1) * 512]

        def im2_ap(b, h):
            if b == 0:
                return im2_b0[h]
            return im2_t[b][:, h * 512:(h + 1) * 512]

        def load_batch_data(b):
            nc.sync.dma_start(out=im2_t[b], in_=d["im2col"][b, :, :].bitcast(F32R))
            for cc in range(NCC):
                nc.sync.dma_start(
                    out=cnn_t[b][cc],
                    in_=d["cnn"][b, cc * 128:(cc + 1) * 128, :].bitcast(F32R),
                )

        load_b0_half(0)
        for _ac in range(1, NAC):
            load_wect(_ac)
        load_b0_half(1)

        # ---- prep compute: fused conv weight + per-batch bias vectors ----
        # WfusedT[t, a] = sum_k Wac_flat[k, t] * WawT[k, a]; rows 121..127
        # are zero because the padded wacf columns are zero.
        wf_ps = apsum.tile([128, A], F32)
        for kc in range(4):
            nc.tensor.matmul(
                wf_ps, lhsT=wacf[:, kc, :], rhs=wawt[:, kc, :],
                start=(kc == 0), stop=(kc == 3),
            )
        wft = const.tile([128, A], F32R)
        nc.vector.tensor_copy(wft, wf_ps)

        # bias bb[p, ac] = bh + bec ; qb[p, ac, b] = (hidden @ Wh.T)[b, a] + bb
        bb = const.tile([128, NAC], F32)
        nc.vector.tensor_add(bb, bhv, becv)
        qb = const.tile([128, NAC, NB], F32)
        for ac in range(NAC):
            q_ps = apsum.tile([128, NB], F32, tag="qps")
            for hc in range(NHC):
                nc.tensor.matmul(
                    q_ps, lhsT=wht[:, hc, ac * 128:(ac + 1) * 128],
                    rhs=hid[:, hc, :], start=(hc == 0), stop=(hc == NHC - 1),
                )
            nc.vector.tensor_scalar_add(qb[:, ac, :], q_ps, bb[:, ac:ac + 1])

        # ---- bulk loads for remaining batch items (sync queue) ----
        for b in range(1, NB):
            load_batch_data(b)

        # ---- main loop ----
        for b in range(NB):
            mask_row = maskp.tile([1, NPIX], F32, tag="mask")
            nc.gpsimd.dma_start(out=mask_row, in_=d["mask"][b, :])
            asum_row = asump.tile([1, NPIX], F32, tag="asum_in")
            nc.gpsimd.dma_start(out=asum_row, in_=d["asum"][b, :])
            em_row = rows2.tile([1, NPIX], F32, tag="em")
            dens = rows1.tile([1, 2], F32, tag="dens")
            ctxc = cpool.tile([128, NCC, 2], F32, tag="ctxc")
            for h in range(2):
                hsl = slice(h * 512, (h + 1) * 512)
                e_ps = epool.tile([1, 512], F32, tag="eps")
                pss = [mpool.tile([128, 512], F32, tag="mps", name=f"ps{ac}")
                       for ac in range(NAC)]
                for cc in range(NCC):
                    for ac in range(NAC):
                        nc.tensor.matmul(
                            pss[ac], lhsT=wect_ac[ac][:, cc, :],
                            rhs=cnn_ap(b, cc, h),
                            start=(cc == 0), stop=False,
                        )
                for ac in range(NAC):
                    nc.tensor.matmul(
                        pss[ac], lhsT=wft[:, ac * 128:(ac + 1) * 128],
                        rhs=im2_ap(b, h), start=False, stop=True,
                    )
                scs = []
                for ac in range(NAC):
                    sc = spool.tile([128, 512], F32R, tag="sc", name=f"sc{ac}")
                    nc.scalar.activation(
                        sc, pss[ac], AF.Tanh, bias=qb[:, ac, b:b + 1], scale=1.0
                    )
                    scs.append(sc)
                for ac in range(NAC):
                    nc.tensor.matmul(
                        e_ps, lhsT=wv[:, ac:ac + 1], rhs=scs[ac],
                        start=(ac == 0), stop=(ac == NAC - 1),
                    )
                # energy -> exp (no max-subtract needed; |energy| is O(1))
                exp_h = rows2.tile([1, 512], F32, tag="exph")
                nc.scalar.activation(exp_h, e_ps, AF.Exp)
                # masked exp + this half's denominator (fused)
                nc.vector.scalar_tensor_tensor(
                    out=em_row[:, hsl], in0=exp_h, scalar=1.0,
                    in1=mask_row[:, hsl], op0=MULT, op1=MULT,
                    accum_out=dens[:, h:h + 1],
                )
                # unnormalized context for this half (overlaps next half's PE)
                ab = bpool.tile([128, 512], F32, tag="ab")
                nc.gpsimd.partition_broadcast(ab, em_row[:, hsl])
                trash = tpool.tile([128, 512], F32, tag="trash")
                for cc in range(NCC):
                    nc.vector.scalar_tensor_tensor(
                        out=trash, in0=cnn_ap(b, cc, h).bitcast(F32),
                        scalar=1.0, in1=ab, op0=MULT, op1=MULT,
                        accum_out=ctxc[:, cc, h:h + 1],
                    )

            # denominator and 1/(den + 1e-10)
            den = rows1.tile([1, 1], F32, tag="den")
            nc.vector.tensor_scalar_add(den, dens[:, 0:1], dens[:, 1:2])
            nc.vector.tensor_scalar_add(den, den, 1e-10)
            rcp = rows1.tile([1, 1], F32, tag="rcp")
            nc.vector.reciprocal(rcp, den)

            # alpha = em * rcp ; alpha_sum_new = alpha + alpha_sum (in place)
            alpha_row = rows2.tile([1, NPIX], F32, tag="alpha")
            nc.vector.tensor_scalar_mul(alpha_row, em_row, rcp)
            nc.sync.dma_start(out=d["alpha"][b, :], in_=alpha_row)
            nc.vector.tensor_add(asum_row, alpha_row, asum_row)
            nc.sync.dma_start(out=d["asum_new"][b, :], in_=asum_row)

            # context: combine halves, scale by rcp (broadcast to partitions)
            rcp128 = rows1.tile([128, 1], F32, tag="rcp128")
            nc.gpsimd.partition_broadcast(rcp128, rcp)
            ctx_fin = cpool.tile([128, NCC], F32, tag="ctxfin")
            nc.vector.tensor_add(ctx_fin, ctxc[:, :, 0], ctxc[:, :, 1])
            nc.vector.tensor_scalar_mul(ctx_fin, ctx_fin, rcp128)
            nc.sync.dma_start(
                out=d["ctx"][b, 0:640].rearrange("(cc p) -> p cc", p=128),
                in_=ctx_fin[:, 0:5],
            )
            nc.sync.dma_start(out=d["ctx"][b, 640:C], in_=ctx_fin[0:CREM, 5:6])


def _build():
    nc = bacc.Bacc(
        "TRN2", target_bir_lowering=False, debug=False, enable_asserts=False
    )
    d = {}
    def inp(name, shape):
        d[name] = nc.dram_tensor(name, list(shape), F32, kind="ExternalInput")[:]
    def outp(name, shape):
        d[name] = nc.dram_tensor(name, list(shape), F32, kind="ExternalOutput")[:]

    inp("cnn", (NB, 768, NPIX))
    inp("im2col", (NB, 128, NPIX))
    inp("hiddent", (HID, NB))
    inp("mask", (NB, NPIX))
    inp("asum", (NB, NPIX))
    inp("wect", (768, A))
    inp("wht", (HID, A))
    inp("wawt", (CK, A))
    inp("wacf", (CK, 128))
    inp("wv", (A,))
    inp("bh", (A,))
    inp("bec", (A,))
    outp("ctx", (NB, C))
    outp("alpha", (NB, NPIX))
    outp("asum_new", (NB, NPIX))

    with tile.TileContext(nc) as tc:
        _emit(nc, tc, d)
    nc.compile()
    return nc


def get_program():
    global _PROGRAM
    if _PROGRAM is None:
        _PROGRAM = _build()
    return _PROGRAM


def _prep_in_maps(cnn_features, hidden, alpha_sum, image_mask,
                  Wh, bh, Wec, bec, Wac, Waw, Wv, bv):
    f = np.float32
    cnn = np.zeros((B, 768, NPIX), dtype=f)
    cnn[:, :C, :] = np.asarray(cnn_features, dtype=f).reshape(B, C, NPIX)
    asum = np.ascontiguousarray(alpha_sum, dtype=f).reshape(B, NPIX)
    mask = np.ascontiguousarray(image_mask, dtype=f).reshape(B, NPIX)
    # host im2col: pure data rearrangement of alpha_sum (zero-padded windows),
    # K-padded from 121 to 128 rows with zeros
    pad = np.zeros((B, H + K - 1, W + K - 1), dtype=f)
    pad[:, K // 2:K // 2 + H, K // 2:K // 2 + W] = asum.reshape(B, H, W)
    win = np.lib.stride_tricks.sliding_window_view(pad, (H, W), axis=(1, 2))
    im2 = np.zeros((B, 128, NPIX), dtype=f)
    im2[:, :KK, :] = win.reshape(B, KK, NPIX)
    wect_pad = np.zeros((768, A), dtype=f)
    wect_pad[:C, :] = np.asarray(Wec, dtype=f).T
    # Wac flat, K-padded 121 -> 128 with zero columns
    wacf = np.zeros((CK, 128), dtype=f)
    wacf[:, :KK] = np.asarray(Wac, dtype=f).reshape(CK, KK)

    shared = {
        "wect": wect_pad,
        "wht": np.ascontiguousarray(np.asarray(Wh, dtype=f).T),
        "wawt": np.ascontiguousarray(np.asarray(Waw, dtype=f).T),
        "wacf": wacf,
        "wv": np.ascontiguousarray(np.asarray(Wv, dtype=f).reshape(A)),
        "bh": np.ascontiguousarray(np.asarray(bh, dtype=f)),
        "bec": np.ascontiguousarray(np.asarray(bec, dtype=f)),
    }
    hiddenT = np.ascontiguousarray(np.asarray(hidden, dtype=f).T)  # [HID, B]
    in_maps = []
    for m in range(NCORES):
        sl = slice(m * NB, (m + 1) * NB)
        in_maps.append({
            "cnn": np.ascontiguousarray(cnn[sl]),
            "im2col": np.ascontiguousarray(im2[sl]),
            "hiddent": np.ascontiguousarray(hiddenT[:, sl]),
            "mask": np.ascontiguousarray(mask[sl]),
            "asum": np.ascontiguousarray(asum[sl]),
            **shared,
        })
    return in_maps


def kernel(cnn_features, hidden, alpha_sum, image_mask,
           Wh, bh, Wec, bec, Wac, Waw, Wv, bv):
    global LAST_RESULT
    nc = get_program()
    in_maps = _prep_in_maps(cnn_features, hidden, alpha_sum, image_mask,
                            Wh, bh, Wec, bec, Wac, Waw, Wv, bv)
    res = run_bass_kernel_spmd(nc, in_maps, list(range(NCORES)))
    LAST_RESULT = res
    ctx = np.concatenate([res.results[m]["ctx"] for m in range(NCORES)], axis=0)
    alpha = np.concatenate(
        [res.results[m]["alpha"] for m in range(NCORES)], axis=0
    ).reshape(B, H, W)
    asum_new = np.concatenate(
        [res.results[m]["asum_new"] for m in range(NCORES)], axis=0
    ).reshape(B, 1, H, W)
    return (ctx.astype(np.float32), alpha.astype(np.float32),
            asum_new.astype(np.float32))
